# revision 1
# baseline (speedup 1.0000x reference)
"""Multi-head attention (16 heads, D=1024, B=2, S=2048) on 8 Trainium2 cores.

Sharding: batch (2) x head-groups (4 heads each) = 8 cores, no collectives.
Each core computes, for its batch b and head group g:
  - Q/K/V projections restricted to the group's 256 output dims
  - per-head attention with masked softmax (mask + 1/sqrt(32) scale folded
    into a single exp activation; no max-subtraction needed since scores are
    small and bounded)
  - partial output = concat(head outs) @ Wo[rows of group g]
Host sums the 4 per-group partials for each batch.

Device layout trick: the host passes X^T (feature-major) so every matmul
chains naturally with zero on-device transposes:
  X^T --(Wq/Wk stationary)--> Q^T,K^T [j, s]
  K^T.T @ Q^T = scores^T [k, q]  --exp-->  E^T
  V is produced in natural [s, j] layout with an interleaved ones column,
  so V'.T @ E^T accumulates attn-weighted V AND the softmax denominator
  (row 64) in one PSUM accumulation group.

All matmuls run in float32r (the PE's 1-cycle/row fp32 mode; plain fp32 is
4 cycles/row). Projections are emitted in 4 interleaved K/Q/V column-rounds
and K^T/Q^T/Oc^T are split into per-chunk tiles so attention/output phases
start as soon as their actual inputs exist.
"""
import ml_dtypes
import numpy as np

import concourse.bacc as bacc
import concourse.mybir as mybir
import concourse.tile as tile
from concourse.bass_utils import run_bass_kernel_spmd

F32 = mybir.dt.float32
F32R = mybir.dt.float32r
BF16 = mybir.dt.bfloat16
AF = mybir.ActivationFunctionType

S = 2048          # sequence length
D = 1024          # model dim
HLOC = 4          # heads per core
HD = 64           # head dim
JG = HLOC * 65    # V-natural tile width (64 data cols + 1 ones col per head)
SCALE = 1.0 / np.sqrt(32.0)   # reference bug: d_k = B*H = 32
MASK_VALUE = -1.0e6

ND = 8            # d chunks of 128 (contraction for projections)
NSC = 4           # s chunks of 512 (projection rounds)
NST = 16          # s tiles of 128
NKC = 16          # k chunks of 128
NQC = 2           # q chunks of 1024
QW = 1024         # q chunk width

_cached_nc = None
LAST_RESULTS = None


def _build():
    nc = bacc.Bacc("TRN2", target_bir_lowering=False, debug=False,
                   num_swdge_queues=4)

    xqT = nc.dram_tensor("xqT", [D, S], BF16, kind="ExternalInput")
    xkT = nc.dram_tensor("xkT", [D, S], BF16, kind="ExternalInput")
    xvT = nc.dram_tensor("xvT", [D, S], BF16, kind="ExternalInput")
    wq = nc.dram_tensor("wq", [D, 256], BF16, kind="ExternalInput")
    wk = nc.dram_tensor("wk", [D, 256], BF16, kind="ExternalInput")
    wv = nc.dram_tensor("wv", [D, 256], BF16, kind="ExternalInput")
    wo = nc.dram_tensor("wo", [256, D], F32R, kind="ExternalInput")
    maskb = nc.dram_tensor("maskb", [128, NKC], F32, kind="ExternalInput")
    out = nc.dram_tensor("out", [S, D], F32, kind="ExternalOutput")

    with tile.TileContext(nc) as tc:
        with tc.tile_pool(name="wp", bufs=1) as wp, \
             tc.tile_pool(name="per", bufs=1) as per, \
             tc.tile_pool(name="xp", bufs=16) as xp, \
             tc.tile_pool(name="ep", bufs=4) as ep, \
             tc.tile_pool(name="unp", bufs=8) as unp, \
             tc.tile_pool(name="rbp", bufs=8) as rbp, \
             tc.tile_pool(name="smol", bufs=1) as smol, \
             tc.tile_pool(name="outp", bufs=6) as outp, \
             tc.tile_pool(name="pj", bufs=2, space="PSUM") as pj, \
             tc.tile_pool(name="psc", bufs=2, space="PSUM") as psc, \
             tc.tile_pool(name="po", bufs=2, space="PSUM") as po:

            # ---- mask + packed projection weights (one 1MB DMA per W) ----
            mt = wp.tile([128, NKC], F32, name="mt", tag="mt")
            nc.sync.dma_start(out=mt, in_=maskb[:, :])
            wk_p = wp.tile([128, ND * 256], BF16, name="wk_p", tag="wk_p")
            wq_p = wp.tile([128, ND * 256], BF16, name="wq_p", tag="wq_p")
            wv_p = wp.tile([128, ND * 256], BF16, name="wv_p", tag="wv_p")
            nc.sync.dma_start(out=wk_p.rearrange("p (n j) -> p n j", j=256),
                              in_=wk.rearrange("(n p) j -> p n j", p=128))
            nc.gpsimd.dma_start(out=wv_p.rearrange("p (n j) -> p n j", j=256),
                                in_=wv.rearrange("(n p) j -> p n j", p=128))
            wk_t = [wk_p[:, d * 256:(d + 1) * 256] for d in range(ND)]
            wq_t = [wq_p[:, d * 256:(d + 1) * 256] for d in range(ND)]
            wv_t = [wv_p[:, d * 256:(d + 1) * 256] for d in range(ND)]
            # exp table preload: a 1-element exp so the ~2.7us ACT table
            # load happens during the projection lead-in, not mid-pipeline
            scr1 = wp.tile([1, 1], F32, name="scr1", tag="scr1")
            nc.scalar.activation(scr1, mt[0:1, 0:1], AF.Exp)

            # ---- persistent activations (chunked for dep granularity) ----
            KTt = [[per.tile([128, 512], F32R, name=f"KT{j}_{s_}",
                             tag=f"KT{j}_{s_}") for s_ in range(NSC)]
                   for j in range(2)]
            QTt = [[per.tile([128, 512], F32R, name=f"QT{j}_{s_}",
                             tag=f"QT{j}_{s_}") for s_ in range(NSC)]
                   for j in range(2)]
            Vn = [per.tile([128, JG], F32R, name=f"Vn{i}", tag=f"Vn{i}")
                  for i in range(NST)]
            OcT = [[per.tile([128, 512], F32R, name=f"OcT{j}_{q}",
                             tag=f"OcT{j}_{q}") for q in range(2 * NQC)]
                   for j in range(2)]

            def k_or_q_round(nm, xdram, wt, OUT, r):
                c0 = r * 512
                xt = [xp.tile([128, 512], BF16, name=f"x{nm}{r}_{d}",
                              tag="xin") for d in range(ND)]
                for d in range(ND):
                    nc.sync.dma_start(
                        out=xt[d],
                        in_=xdram[d * 128:(d + 1) * 128, c0:c0 + 512])
                for jt in range(2):
                    pt = pj.tile([128, 512], F32, name=f"p{nm}{r}_{jt}",
                                 tag="pj")
                    for d in range(ND):
                        nc.tensor.matmul(
                            pt, wt[d][:, jt * 128:(jt + 1) * 128],
                            xt[d], start=(d == 0), stop=(d == ND - 1))
                    nc.vector.tensor_copy(OUT[jt][r], pt)

            def v_round(r):
                c0 = r * 512
                xvt = [xp.tile([128, 512], BF16, name=f"xv{r}_{d}", tag="xin")
                       for d in range(ND)]
                for d in range(ND):
                    nc.gpsimd.dma_start(
                        out=xvt[d],
                        in_=xvT[d * 128:(d + 1) * 128, c0:c0 + 512])
                for stl in range(4):
                    st = r * 4 + stl
                    pt = pj.tile([128, 256], F32, name=f"pv{st}", tag="pj")
                    for d in range(ND):
                        nc.tensor.matmul(
                            pt, xvt[d][:, stl * 128:(stl + 1) * 128], wv_t[d],
                            start=(d == 0), stop=(d == ND - 1))
                    vt = Vn[st]
                    vspl = vt.rearrange("p (h x) -> p h x", x=65)
                    nc.vector.memset(vspl[:, :, 64:65].bitcast(F32), 1.0)
                    nc.vector.tensor_copy(
                        vspl[:, :, 0:64],
                        pt.rearrange("p (h j) -> p h j", j=64))

            def attention_head(qc, h, seg_hook=None, tail_head=False):
                jt, hr = divmod(h, 2)
                hoff = hr * 64
                pots = [po.tile([65, 512], F32, name=f"pot{qc}_{h}_{qh}",
                                tag="pot") for qh in range(2)]
                for kc in range(NKC):
                    if seg_hook is not None and kc % 4 == 0:
                        seg_hook(kc)
                    ks, ko = divmod(kc, 4)
                    pst = psc.tile([128, QW], F32,
                                   name=f"pst{qc}_{h}_{kc}", tag="pst")
                    for qh in range(2):
                        nc.tensor.matmul(
                            pst[:, qh * 512:(qh + 1) * 512],
                            KTt[jt][ks][hoff:hoff + 64,
                                        ko * 128:(ko + 1) * 128],
                            QTt[jt][2 * qc + qh][hoff:hoff + 64, :],
                            start=True, stop=True)
                    et = ep.tile([128, QW], F32R,
                                 name=f"et{qc}_{h}_{kc}", tag="et")
                    nc.scalar.activation(et, pst, AF.Exp,
                                         bias=mt[:, kc:kc + 1],
                                         scale=float(SCALE))
                    for qh in range(2):
                        nc.tensor.matmul(
                            pots[qh],
                            Vn[kc][:, h * 65:h * 65 + 65],
                            et[:, qh * 512:(qh + 1) * 512],
                            start=(kc == 0), stop=(kc == NKC - 1))
                # drain + normalize each q-half independently: the first
                # half's chain (and its PSUM bank) overlaps the second
                # half's tail, and the output projection unblocks per half
                for qh in range(2):
                    un = unp.tile([65, 512], F32, name=f"un{qc}_{h}_{qh}",
                                  tag="un")
                    dtmp = rbp.tile([1, 512], F32, name=f"dt{qc}_{h}_{qh}",
                                    tag="tmp1")
                    if tail_head:
                        # very last head: ACT is idle, so drain on ACT while
                        # DVE stages the denom row straight from PSUM --
                        # shortens the serial chain before the final wo tiles
                        nc.scalar.copy(un, pots[qh][:, :])
                        nc.vector.tensor_copy(dtmp, pots[qh][64:65, :])
                    else:
                        nc.vector.tensor_copy(un, pots[qh][:, :])
                        # reciprocal_approx_* reads garbage at a nonzero
                        # partition offset: stage the denom row at part. 0
                        nc.vector.tensor_copy(dtmp, un[64:65, :])
                    rrow = rbp.tile([1, 512], F32, name=f"rr{qc}_{h}_{qh}",
                                    tag="tmp1")
                    rsc1 = rbp.tile([1, 512], F32, name=f"rs{qc}_{h}_{qh}",
                                    tag="tmp1")
                    nc.vector.reciprocal_approx_accurate(rrow, dtmp, rsc1)
                    rb = rbp.tile([64, 512], F32, name=f"rb{qc}_{h}_{qh}",
                                  tag="rb")
                    nc.gpsimd.partition_broadcast(rb, rrow[0:1, :])
                    nc.vector.tensor_mul(
                        OcT[jt][2 * qc + qh][hoff:hoff + 64, :],
                        un[0:64, :], rb)

            def wo_phase(sts, tail):
                for i, st in enumerate(sts):
                    sq, so = divmod(st, 4)
                    for ec in range(2):
                        pool = psc if (tail and (i + ec) % 2 == 0) else pj
                        ptag = "pst" if pool is psc else "pj"
                        pt = pool.tile([128, 512], F32, name=f"pw{st}_{ec}",
                                       tag=ptag)
                        for jc in range(2):
                            nc.tensor.matmul(
                                pt, OcT[jc][sq][:, so * 128:(so + 1) * 128],
                                wo_t[jc][:, ec * 512:(ec + 1) * 512],
                                start=(jc == 0), stop=(jc == 1))
                        ot = outp.tile([128, 512], F32, name=f"ot{st}_{ec}",
                                       tag="ot")
                        if tail and ec == 0:
                            nc.scalar.copy(ot, pt)
                        else:
                            nc.vector.tensor_copy(ot, pt)
                        nc.sync.dma_start(
                            out=out[st * 128:(st + 1) * 128,
                                    ec * 512:(ec + 1) * 512],
                            in_=ot)

            # ---- emission schedule ----
            # lead-in: exactly what attention(qc0, h0, kc0..3) needs, first
            k_or_q_round("k", xkT, wk_t, KTt, 0)
            nc.sync.dma_start(out=wq_p.rearrange("p (n j) -> p n j", j=256),
                              in_=wq.rearrange("(n p) j -> p n j", p=128))
            k_or_q_round("q", xqT, wq_t, QTt, 0)
            k_or_q_round("q", xqT, wq_t, QTt, 1)
            v_round(0)

            def h0_hook(kc):
                # stream the remaining K/V rounds in just ahead of the
                # segments of head 0 that consume them
                if kc == 4:
                    k_or_q_round("k", xkT, wk_t, KTt, 1)
                    v_round(1)
                elif kc == 8:
                    k_or_q_round("k", xkT, wk_t, KTt, 2)
                    v_round(2)
                elif kc == 12:
                    k_or_q_round("k", xkT, wk_t, KTt, 3)
                    v_round(3)

            attention_head(0, 0, seg_hook=h0_hook)
            for h in range(1, HLOC):
                attention_head(0, h)

            wo_p = wp.tile([128, 2 * D], F32R, name="wo_p", tag="wo_p")
            nc.sync.dma_start(out=wo_p.rearrange("p (n j) -> p n j", j=D),
                              in_=wo.rearrange("(n p) j -> p n j", p=128))
            wo_t = [wo_p[:, j * D:(j + 1) * D] for j in range(2)]

            k_or_q_round("q", xqT, wq_t, QTt, 2)
            k_or_q_round("q", xqT, wq_t, QTt, 3)
            for h in range(HLOC):
                attention_head(1, h, tail_head=(h == HLOC - 1))
            wo_phase(range(0, 8), False)   # qc0: runs under attention(qc1)
            wo_phase(range(8, 16), True)   # qc1: tail, ACT idle, more psum
    nc.compile()
    return nc


def _get_nc():
    global _cached_nc
    if _cached_nc is None:
        _cached_nc = _build()
    return _cached_nc


def kernel(queries, keys, values, valid_lens, Wq, Wk, Wv, Wo, **kwargs):
    queries = np.asarray(queries, dtype=np.float32)
    keys = np.asarray(keys, dtype=np.float32)
    values = np.asarray(values, dtype=np.float32)
    Wq = np.asarray(Wq, dtype=np.float32)
    Wk = np.asarray(Wk, dtype=np.float32)
    Wv = np.asarray(Wv, dtype=np.float32)
    Wo = np.asarray(Wo, dtype=np.float32)
    vls = np.asarray(valid_lens).astype(np.int64)
    B = queries.shape[0]
    assert B == 2 and queries.shape[1:] == (S, D), \
        f"kernel compiled for (2, {S}, {D}), got {queries.shape}"

    nc = _get_nc()

    in_maps = []
    for b in range(B):
        vl = int(vls[b])
        qb = queries[b]
        if vl <= 0:
            # reference: fully-masked row -> softmax of constant -> uniform.
            # Zero queries give zero scores -> uniform attention, and an
            # all-zero mask keeps every position in the denominator.
            qb = np.zeros_like(qb)
            mk = np.zeros(S, np.float32)
        else:
            mk = np.where(np.arange(S) < vl, 0.0, MASK_VALUE).astype(np.float32)
        mkt = np.ascontiguousarray(mk.reshape(NKC, 128).T)  # [128, NKC]
        bf16 = ml_dtypes.bfloat16
        xq = np.ascontiguousarray(qb.T).astype(bf16)
        xk = np.ascontiguousarray(keys[b].T).astype(bf16)
        xv = np.ascontiguousarray(values[b].T).astype(bf16)
        for g in range(4):
            in_maps.append({
                "xqT": xq, "xkT": xk, "xvT": xv,
                "wq": np.ascontiguousarray(Wq[:, g * 256:(g + 1) * 256]).astype(bf16),
                "wk": np.ascontiguousarray(Wk[:, g * 256:(g + 1) * 256]).astype(bf16),
                "wv": np.ascontiguousarray(Wv[:, g * 256:(g + 1) * 256]).astype(bf16),
                "wo": np.ascontiguousarray(Wo[g * 256:(g + 1) * 256, :]),
                "maskb": mkt,
            })

    res = run_bass_kernel_spmd(nc, in_maps, core_ids=list(range(8)), **kwargs)
    global LAST_RESULTS
    LAST_RESULTS = res

    outp = np.zeros((B, S, D), np.float32)
    for b in range(B):
        acc = res.results[b * 4 + 0]["out"].astype(np.float32)
        for g in range(1, 4):
            acc = acc + res.results[b * 4 + g]["out"]
        outp[b] = acc
    return outp



# revision 3
# speedup vs baseline: 2.2396x; 2.2396x over previous
"""Multi-head attention (16 heads, D=1024, B=2, S=2048) on 8 Trainium2 cores.

Sharding v2: head-wise tensor parallel — each core owns 2 heads (128 of the
1024 projection dims) and processes BOTH batches.  Per-core partial outputs
(full [2, 2048, 1024] shape through its 128 rows of Wo) are summed on host.

Key optimization vs v1: `valid_lens` is known at kernel-build time and masks
all scores at k >= vl to exp(-1e6) == 0 exactly, so k-chunks beyond
ceil(vl/128) contribute nothing to numerator or denominator.  The kernel is
compiled per (nk0, nk1) = ceil(vl/128) and never computes the masked
K/V projections, scores, exps, or AV products.  With vl=[288, 576] that cuts
attention work 4x and K/V projection work 3.2x, and head-wise sharding keeps
all 8 cores perfectly balanced (each sees both batches).

Device layout (per core, per batch b):
  X^T (feature-major, bf16) --(Wq/Wk stationary)--> QT/KT [j=128 dims, s]
  KT.T @ QT = scores^T [k, q] --exp(scale*x + mask)--> E [k, q]  (f32r)
  Vn natural [k, 2*65] (64 dims + ones col per head):
    Vn_h.T @ E accumulates attn-weighted V AND the softmax denominator
    (row 64) in one PSUM accumulation group.
  normalize: OcT[h*64:, q] = pots[0:64] * broadcast(1/pots[64])
  wo: OcT chunk [128, 128] stationary x Wo rows [128, 1024] -> out partial.

All attention matmuls run f32r at 1 cycle/row (free dim >= 256; the V/K-proj
tails with free 128 are bf16-input).  Outputs are written bf16 to halve the
output DMA; host accumulates the 8 partials in f32.
"""
import ml_dtypes
import numpy as np

import concourse.bacc as bacc
import concourse.mybir as mybir
import concourse.tile as tile
from concourse.bass_utils import run_bass_kernel_spmd

F32 = mybir.dt.float32
F32R = mybir.dt.float32r
BF16 = mybir.dt.bfloat16
AF = mybir.ActivationFunctionType

S = 2048          # sequence length
D = 1024          # model dim
HLOC = 2          # heads per core
HD = 64           # head dim
SCALE = 1.0 / np.sqrt(32.0)   # reference bug: d_k = B*H = 32
MASK_VALUE = -1.0e6

ND = 8            # d chunks of 128 (contraction for projections)
NQS = 4           # q chunks of 512 per batch (OcT/QT chunk granularity)

_cached = {}
LAST_RESULTS = None


def _build(nk0, nk1):
    nks = [nk0, nk1]
    nc = bacc.Bacc("TRN2", target_bir_lowering=False, debug=False,
                   num_swdge_queues=4)

    xq = [nc.dram_tensor(f"xq{b}", [D, S], BF16, kind="ExternalInput")
          for b in range(2)]
    xk = [nc.dram_tensor(f"xk{b}", [D, nks[b] * 128], BF16,
                         kind="ExternalInput") for b in range(2)]
    xv = [nc.dram_tensor(f"xv{b}", [D, nks[b] * 128], BF16,
                         kind="ExternalInput") for b in range(2)]
    wq = nc.dram_tensor("wq", [D, 128], BF16, kind="ExternalInput")
    wk = nc.dram_tensor("wk", [D, 128], BF16, kind="ExternalInput")
    wv = nc.dram_tensor("wv", [D, 128], BF16, kind="ExternalInput")
    wo = nc.dram_tensor("wo", [128, D], F32R, kind="ExternalInput")
    maskb = nc.dram_tensor("maskb", [128, 2], F32, kind="ExternalInput")
    out = [nc.dram_tensor(f"out{b}", [S, D], BF16, kind="ExternalOutput")
           for b in range(2)]

    with tile.TileContext(nc) as tc:
        with tc.tile_pool(name="wp", bufs=1) as wp, \
             tc.tile_pool(name="per", bufs=1) as per, \
             tc.tile_pool(name="xp", bufs=2) as xp, \
             tc.tile_pool(name="kvp", bufs=3) as kvp, \
             tc.tile_pool(name="ep", bufs=3) as ep, \
             tc.tile_pool(name="rbp", bufs=8) as rbp, \
             tc.tile_pool(name="outp", bufs=6) as outp, \
             tc.tile_pool(name="pp", bufs=2, space="PSUM") as pp, \
             tc.tile_pool(name="po", bufs=2, space="PSUM") as po, \
             tc.tile_pool(name="psc", bufs=2, space="PSUM") as psc:

            # ---- mask + packed projection weights ----
            # (wk first on SP and xk0 first on Pool: the K projection is the
            # head of the whole pipeline)
            wk_p = wp.tile([128, ND * 128], BF16, name="wk_p", tag="wk_p")
            wq_p = wp.tile([128, ND * 128], BF16, name="wq_p", tag="wq_p")
            wv_p = wp.tile([128, ND * 128], BF16, name="wv_p", tag="wv_p")
            nc.scalar.dma_start(out=wk_p.rearrange("p (n j) -> p n j", j=128),
                                in_=wk.rearrange("(n p) j -> p n j", p=128))
            mt = wp.tile([128, 2], F32, name="mt", tag="mt")
            nc.scalar.dma_start(out=mt, in_=maskb[:, :])
            nc.scalar.dma_start(out=wq_p.rearrange("p (n j) -> p n j", j=128),
                                in_=wq.rearrange("(n p) j -> p n j", p=128))
            wk_t = [wk_p[:, d * 128:(d + 1) * 128] for d in range(ND)]
            wq_t = [wq_p[:, d * 128:(d + 1) * 128] for d in range(ND)]
            wv_t = [wv_p[:, d * 128:(d + 1) * 128] for d in range(ND)]
            # exp table preload: a 1-element exp so the ~2.7us ACT table
            # load happens during the projection lead-in, not mid-pipeline
            scr1 = wp.tile([1, 1], F32, name="scr1", tag="scr1")
            nc.scalar.activation(scr1, mt[0:1, 0:1], AF.Exp)

            # ---- persistent activations (chunked for dep granularity) ----
            def kcols(b):
                return nks[b] * 128

            def round_widths(total):
                w = []
                while total > 0:
                    w.append(min(512, total))
                    total -= w[-1]
                return w

            KTt = [[per.tile([128, w], F32R, name=f"KT{b}_{i}",
                             tag=f"KT{b}_{i}")
                    for i, w in enumerate(round_widths(kcols(b)))]
                   for b in range(2)]
            QTt = [[per.tile([128, 512], F32R, name=f"QT{b}_{r}",
                             tag=f"QT{b}_{r}") for r in range(NQS)]
                   for b in range(2)]
            Vn = [[per.tile([128, HLOC * 65], F32R, name=f"Vn{b}_{i}",
                            tag=f"Vn{b}_{i}") for i in range(nks[b])]
                  for b in range(2)]
            OcT = [[per.tile([128, 512], F32R, name=f"OcT{b}_{q}",
                             tag=f"OcT{b}_{q}") for q in range(NQS)]
                   for b in range(2)]

            def kt_slice(b, kc):
                """KT stationary slice [*, kc*128:(kc+1)*128] across tiles."""
                c0 = kc * 128
                ti, off = divmod(c0, 512)
                return KTt[b][ti][:, off:off + 128]

            def kq_round(nm, xdram, wt, OUT, b, r):
                """One 512-wide Q projection round for batch b."""
                c0 = r * 512
                xt = xp.tile([128, ND * 512], BF16, name=f"x{nm}{b}_{r}",
                             tag="xin")
                nc.sync.dma_start(
                    out=xt.rearrange("p (n s) -> p n s", n=ND),
                    in_=xdram[:, c0:c0 + 512]
                    .rearrange("(n p) s -> p n s", p=128))
                xts = xt.rearrange("p (n s) -> p n s", n=ND)
                pt = pp.tile([128, 512], F32, name=f"p{nm}{b}_{r}", tag="pp")
                for d in range(ND):
                    nc.tensor.matmul(
                        pt, wt[d], xts[:, d, :],
                        start=(d == 0), stop=(d == ND - 1))
                nc.vector.tensor_copy(OUT, pt)

            def k_rounds(b):
                """K^T projection for all nk_b k-chunks of batch b."""
                w = kcols(b)
                xkt = kvp.tile([128, ND * w], BF16, name=f"xk{b}", tag="xkv")
                xks = xkt.rearrange("p (n s) -> p n s", n=ND)
                xkd = xk[b].rearrange("(n p) s -> p n s", p=128)
                for half in range(2):
                    dsl = slice(half * (ND // 2), (half + 1) * (ND // 2))
                    nc.gpsimd.dma_start(out=xks[:, dsl, :],
                                        in_=xkd[:, dsl, :])
                for i, rw in enumerate(round_widths(w)):
                    c0 = i * 512
                    pt = pp.tile([128, rw], F32, name=f"pk{b}_{i}", tag="pp")
                    for d in range(ND):
                        nc.tensor.matmul(
                            pt, wk_t[d], xks[:, d, c0:c0 + rw],
                            start=(d == 0), stop=(d == ND - 1))
                    nc.vector.tensor_copy(KTt[b][i], pt)

            wv_loaded = [False]

            def v_round(b):
                """V natural-layout projection for all k-chunks of batch b."""
                if not wv_loaded[0]:
                    wv_loaded[0] = True
                    nc.gpsimd.dma_start(
                        out=wv_p.rearrange("p (n j) -> p n j", j=128),
                        in_=wv.rearrange("(n p) j -> p n j", p=128))
                w = kcols(b)
                xvt = kvp.tile([128, ND * w], BF16, name=f"xv{b}", tag="xkv")
                nc.gpsimd.dma_start(
                    out=xvt.rearrange("p (n s) -> p n s", n=ND),
                    in_=xv[b].rearrange("(n p) s -> p n s", p=128))
                xvs = xvt.rearrange("p (n s) -> p n s", n=ND)
                for kc in range(nks[b]):
                    pt = pp.tile([128, 128], F32, name=f"pv{b}_{kc}",
                                 tag="pp")
                    for d in range(ND):
                        nc.tensor.matmul(
                            pt, xvs[:, d, kc * 128:(kc + 1) * 128], wv_t[d],
                            start=(d == 0), stop=(d == ND - 1))
                    vt = Vn[b][kc]
                    vspl = vt.rearrange("p (h x) -> p h x", x=65)
                    nc.vector.memset(vspl[:, :, 64:65].bitcast(F32), 1.0)
                    nc.vector.tensor_copy(
                        vspl[:, :, 0:64],
                        pt.rearrange("p (h j) -> p h j", j=64))

            ncopy = [0]

            def spread_copy(dst, src, engines):
                # round-robin big PSUM->SBUF copies across the given engines
                eng = engines[ncopy[0] % len(engines)]
                ncopy[0] += 1
                if eng is nc.scalar:
                    eng.copy(dst, src)
                else:
                    eng.tensor_copy(dst, src)

            fills = []
            in_attn = [False]

            def pop_fill(n=1):
                for _ in range(n):
                    if fills:
                        fills.pop(0)()

            def flush_fills():
                while fills:
                    fills.pop(0)()

            def attention(b, h, qw):
                """One head, one 1024-wide q window of batch b.

                Scores run one kc ahead of AV so the in-order PE queue is
                never parked on an exp dependency; fill thunks (projection
                rounds / wo chunks) are drained between kc steps.
                """
                nk = nks[b]
                in_attn[0] = True
                hoff = h * 64
                pots = [po.tile([65, 512], F32, name=f"pot{b}_{h}_{qw}_{qh}",
                                tag="po") for qh in range(2)]
                ets = [None] * nk

                def scores_exp(kc):
                    pst = psc.tile([128, 1024], F32,
                                   name=f"pst{b}_{h}_{qw}_{kc}", tag="psc")
                    for qh in range(2):
                        nc.tensor.matmul(
                            pst[:, qh * 512:(qh + 1) * 512],
                            kt_slice(b, kc)[hoff:hoff + 64, :],
                            QTt[b][2 * qw + qh][hoff:hoff + 64, :],
                            start=True, stop=True)
                    et = ep.tile([128, 1024], F32R,
                                 name=f"et{b}_{h}_{qw}_{kc}", tag="et")
                    bias = mt[:, b:b + 1] if kc == nk - 1 else 0.0
                    nc.scalar.activation(et, pst, AF.Exp,
                                         bias=bias, scale=float(SCALE))
                    ets[kc] = et

                scores_exp(0)
                for kc in range(nk):
                    if kc + 1 < nk:
                        scores_exp(kc + 1)
                    for qh in range(2):
                        nc.tensor.matmul(
                            pots[qh],
                            Vn[b][kc][:, h * 65:h * 65 + 65],
                            ets[kc][:, qh * 512:(qh + 1) * 512],
                            start=(kc == 0), stop=(kc == nk - 1))
                    pop_fill(2 if len(fills) >= fill_slots[0] else 1)
                    fill_slots[0] = max(1, fill_slots[0] - 1)
                # 1/den can start as soon as the denominator rows are
                # staged; un copies (DVE+ACT) drain pots for Pool's muls
                # (Pool cannot read PSUM)
                dtmp = rbp.tile([1, 1024], F32, name=f"dt{b}{h}{qw}",
                                tag="tmp1")
                for qh in range(2):
                    nc.vector.tensor_copy(dtmp[:, qh * 512:(qh + 1) * 512],
                                          pots[qh][64:65, :])
                rrow = rbp.tile([1, 1024], F32, name=f"rr{b}{h}{qw}",
                                tag="tmp1")
                nc.vector.reciprocal(rrow, dtmp)
                rb = rbp.tile([64, 1024], F32, name=f"rb{b}{h}{qw}", tag="rb")
                nc.gpsimd.partition_broadcast(rb, rrow[0:1, :])
                uns = []
                for qh in range(2):
                    un = rbp.tile([64, 512], F32, name=f"un{b}{h}{qw}{qh}",
                                  tag="un")
                    if qh == 0:
                        nc.vector.tensor_copy(un, pots[qh][0:64, :])
                    else:
                        nc.scalar.copy(un, pots[qh][0:64, :])
                    uns.append(un)
                for qh in range(2):
                    nc.gpsimd.tensor_mul(
                        OcT[b][2 * qw + qh][hoff:hoff + 64, :],
                        uns[qh], rb[:, qh * 512:(qh + 1) * 512])
                in_attn[0] = False

            fill_slots = [0]

            nwo = [0]

            def wo_chunk(b, st, use_psc=False):
                """One 128-row output chunk of batch b's partial product."""
                qs, i = divmod(st, 4)
                ot = outp.tile([128, 1024], BF16, name=f"ot{b}_{st}",
                               tag="ot")
                if use_psc:
                    pt = psc.tile([128, 1024], F32, name=f"pw{b}_{st}",
                                  tag="psc")
                    for odh in range(2):
                        nc.tensor.matmul(
                            pt[:, odh * 512:(odh + 1) * 512],
                            OcT[b][qs][:, i * 128:(i + 1) * 128],
                            wo_t[:, odh * 512:(odh + 1) * 512],
                            start=True, stop=True)
                    s6 = nwo[0] % 6
                    if s6 in ((1, 4) if in_attn[0] else (1, 3, 5)):
                        nc.scalar.copy(ot, pt)
                    else:
                        nc.vector.tensor_copy(ot, pt)
                    nwo[0] += 1
                    eng = nc.sync if (st % 2 == 0) else nc.gpsimd
                    eng.dma_start(
                        out=out[b][st * 128:(st + 1) * 128, :], in_=ot)
                    return
                for odh in range(2):
                    pt = pp.tile([128, 512], F32, name=f"pw{b}_{st}_{odh}",
                                 tag="pp")
                    nc.tensor.matmul(
                        pt,
                        OcT[b][qs][:, i * 128:(i + 1) * 128],
                        wo_t[:, odh * 512:(odh + 1) * 512],
                        start=True, stop=True)
                    # exp saturates ACT inside attention windows: drain
                    # on DVE there, alternate DVE/ACT outside them
                    s6 = nwo[0] % 6
                    if s6 in ((1, 4) if in_attn[0] else (1, 3, 5)):
                        nc.scalar.copy(ot[:, odh * 512:(odh + 1) * 512], pt)
                    else:
                        nc.vector.tensor_copy(
                            ot[:, odh * 512:(odh + 1) * 512], pt)
                    nwo[0] += 1
                eng = nc.sync if (st % 2 == 0) else nc.gpsimd
                eng.dma_start(
                    out=out[b][st * 128:(st + 1) * 128, :], in_=ot)

            # ---- emission schedule ----
            k_rounds(0)
            kq_round("q", xq[0], wq_t, QTt[0][0], 0, 0)
            v_round(0)
            kq_round("q", xq[0], wq_t, QTt[0][1], 0, 1)

            wo_p = wp.tile([128, D], F32R, name="wo_p", tag="wo_p")
            wo_t = wo_p

            # b0 attention, PE fed by remaining projection thunks
            fills.extend([
                lambda: kq_round("q", xq[0], wq_t, QTt[0][2], 0, 2),
                lambda: kq_round("q", xq[0], wq_t, QTt[0][3], 0, 3),
                lambda: k_rounds(1),
                lambda: v_round(1),
                lambda: kq_round("q", xq[1], wq_t, QTt[1][0], 1, 0),
                lambda: kq_round("q", xq[1], wq_t, QTt[1][1], 1, 1),
                lambda: kq_round("q", xq[1], wq_t, QTt[1][2], 1, 2),
                lambda: kq_round("q", xq[1], wq_t, QTt[1][3], 1, 3),
            ])
            fill_slots[0] = 4 * nks[0]
            attention(0, 0, 0)
            attention(0, 1, 0)
            nc.sync.dma_start(out=wo_p, in_=wo[:, :])
            attention(0, 0, 1)
            attention(0, 1, 1)
            flush_fills()

            # b1 attention, PE fed by wo chunks; wo(1, qs) appended once its
            # OcT chunks' producers are emitted.
            fill_slots[0] = 4 * nks[1]
            for st in range(16):
                fills.append(lambda st=st: wo_chunk(0, st))
            attention(1, 0, 0)
            attention(1, 1, 0)
            for st in range(8):
                fills.append(lambda st=st: wo_chunk(1, st))
            attention(1, 0, 1)
            attention(1, 1, 1)
            flush_fills()
            for j, st in enumerate(range(8, 16)):
                wo_chunk(1, st, use_psc=(j % 2 == 0))
    nc.compile()
    return nc


def _get_nc(nk0, nk1):
    key = (nk0, nk1)
    if key not in _cached:
        _cached[key] = _build(nk0, nk1)
    return _cached[key]


def kernel(queries, keys, values, valid_lens, Wq, Wk, Wv, Wo, **kwargs):
    queries = np.asarray(queries, dtype=np.float32)
    keys = np.asarray(keys, dtype=np.float32)
    values = np.asarray(values, dtype=np.float32)
    Wq = np.asarray(Wq, dtype=np.float32)
    Wk = np.asarray(Wk, dtype=np.float32)
    Wv = np.asarray(Wv, dtype=np.float32)
    Wo = np.asarray(Wo, dtype=np.float32)
    vls = np.asarray(valid_lens).astype(np.int64)
    B = queries.shape[0]
    assert B == 2 and queries.shape[1:] == (S, D), \
        f"kernel compiled for (2, {S}, {D}), got {queries.shape}"

    nks = [int(min(16, max(1, -(-int(vls[b]) // 128)))) for b in range(B)]
    nc = _get_nc(nks[0], nks[1])

    bf16 = ml_dtypes.bfloat16
    xqs, xks, xvs, mks = [], [], [], []
    for b in range(B):
        vl = int(vls[b])
        nk = nks[b]
        qb = queries[b]
        if vl <= 0:
            # reference: fully-masked row -> softmax of constant -> uniform.
            # (cannot happen with this reference's randint(1, S+1) bounds)
            qb = np.zeros_like(qb)
            mk = np.zeros(128, np.float32)
        else:
            pos = (nk - 1) * 128 + np.arange(128)
            mk = np.where(pos < vl, 0.0, MASK_VALUE).astype(np.float32)
        mks.append(mk)
        xqs.append(np.ascontiguousarray(qb.T).astype(bf16))
        xks.append(np.ascontiguousarray(keys[b].T[:, :nk * 128]).astype(bf16))
        xvs.append(np.ascontiguousarray(values[b].T[:, :nk * 128]).astype(bf16))
    mkt = np.ascontiguousarray(np.stack(mks, axis=1))  # [128, 2]

    in_maps = []
    for c in range(8):
        sl = slice(c * 128, (c + 1) * 128)
        in_maps.append({
            "xq0": xqs[0], "xq1": xqs[1],
            "xk0": xks[0], "xk1": xks[1],
            "xv0": xvs[0], "xv1": xvs[1],
            "wq": np.ascontiguousarray(Wq[:, sl]).astype(bf16),
            "wk": np.ascontiguousarray(Wk[:, sl]).astype(bf16),
            "wv": np.ascontiguousarray(Wv[:, sl]).astype(bf16),
            "wo": np.ascontiguousarray(Wo[sl, :]),
            "maskb": mkt,
        })

    res = run_bass_kernel_spmd(nc, in_maps, core_ids=list(range(8)), **kwargs)
    global LAST_RESULTS
    LAST_RESULTS = res

    outp = np.zeros((B, S, D), np.float32)
    for b in range(B):
        acc = res.results[0][f"out{b}"].astype(np.float32)
        for c in range(1, 8):
            acc = acc + res.results[c][f"out{b}"].astype(np.float32)
        outp[b] = acc
    return outp


# revision 4
# speedup vs baseline: 2.2735x; 1.0151x over previous
"""Multi-head attention (16 heads, D=1024, B=2, S=2048) on 8 Trainium2 cores.

Sharding v2: head-wise tensor parallel — each core owns 2 heads (128 of the
1024 projection dims) and processes BOTH batches.  Per-core partial outputs
(full [2, 2048, 1024] shape through its 128 rows of Wo) are summed on host.

Key optimization vs v1: `valid_lens` is known at kernel-build time and masks
all scores at k >= vl to exp(-1e6) == 0 exactly, so k-chunks beyond
ceil(vl/128) contribute nothing to numerator or denominator.  The kernel is
compiled per (nk0, nk1) = ceil(vl/128) and never computes the masked
K/V projections, scores, exps, or AV products.  With vl=[288, 576] that cuts
attention work 4x and K/V projection work 3.2x, and head-wise sharding keeps
all 8 cores perfectly balanced (each sees both batches).

Device layout (per core, per batch b):
  X^T (feature-major, bf16) --(Wq/Wk stationary)--> QT/KT [j=128 dims, s]
  KT.T @ QT = scores^T [k, q] --exp(scale*x + mask)--> E [k, q]  (f32r)
  Vn natural [k, 2*65] (64 dims + ones col per head):
    Vn_h.T @ E accumulates attn-weighted V AND the softmax denominator
    (row 64) in one PSUM accumulation group.
  normalize: OcT[h*64:, q] = pots[0:64] * broadcast(1/pots[64])
  wo: OcT chunk [128, 128] stationary x Wo rows [128, 1024] -> out partial.

All attention matmuls run f32r at 1 cycle/row (free dim >= 256; the V/K-proj
tails with free 128 are bf16-input).  Outputs are written bf16 to halve the
output DMA; host accumulates the 8 partials in f32.
"""
import ml_dtypes
import numpy as np

import concourse.bacc as bacc
import concourse.mybir as mybir
import concourse.tile as tile
from concourse.bass_utils import run_bass_kernel_spmd

F32 = mybir.dt.float32
F32R = mybir.dt.float32r
BF16 = mybir.dt.bfloat16
AF = mybir.ActivationFunctionType

S = 2048          # sequence length
D = 1024          # model dim
HLOC = 2          # heads per core
HD = 64           # head dim
SCALE = 1.0 / np.sqrt(32.0)   # reference bug: d_k = B*H = 32
MASK_VALUE = -1.0e6

ND = 8            # d chunks of 128 (contraction for projections)
NQS = 4           # q chunks of 512 per batch (OcT/QT chunk granularity)

_cached = {}
LAST_RESULTS = None


def _build(nk0, nk1):
    nks = [nk0, nk1]
    nc = bacc.Bacc("TRN2", target_bir_lowering=False, debug=False,
                   num_swdge_queues=4)

    xq = [nc.dram_tensor(f"xq{b}", [D, S], BF16, kind="ExternalInput")
          for b in range(2)]
    xk = [nc.dram_tensor(f"xk{b}", [D, nks[b] * 128], BF16,
                         kind="ExternalInput") for b in range(2)]
    xv = [nc.dram_tensor(f"xv{b}", [D, nks[b] * 128], BF16,
                         kind="ExternalInput") for b in range(2)]
    wq = nc.dram_tensor("wq", [D, 128], BF16, kind="ExternalInput")
    wk = nc.dram_tensor("wk", [D, 128], BF16, kind="ExternalInput")
    wv = nc.dram_tensor("wv", [D, 128], BF16, kind="ExternalInput")
    wo = nc.dram_tensor("wo", [128, D], F32R, kind="ExternalInput")
    maskb = nc.dram_tensor("maskb", [128, 2], F32, kind="ExternalInput")
    out = [nc.dram_tensor(f"out{b}", [S, D], BF16, kind="ExternalOutput")
           for b in range(2)]

    with tile.TileContext(nc) as tc:
        with tc.tile_pool(name="wp", bufs=1) as wp, \
             tc.tile_pool(name="per", bufs=1) as per, \
             tc.tile_pool(name="xp", bufs=3) as xp, \
             tc.tile_pool(name="kvp", bufs=2) as kvp, \
             tc.tile_pool(name="ep", bufs=4) as ep, \
             tc.tile_pool(name="rbp", bufs=7) as rbp, \
             tc.tile_pool(name="outp", bufs=8) as outp, \
             tc.tile_pool(name="pp", bufs=2, space="PSUM") as pp, \
             tc.tile_pool(name="po", bufs=2, space="PSUM") as po, \
             tc.tile_pool(name="psc", bufs=2, space="PSUM") as psc:

            # ---- mask + packed projection weights ----
            # (wk first on SP and xk0 first on Pool: the K projection is the
            # head of the whole pipeline)
            wk_p = wp.tile([128, ND * 128], BF16, name="wk_p", tag="wk_p")
            wq_p = wp.tile([128, ND * 128], BF16, name="wq_p", tag="wq_p")
            wv_p = wp.tile([128, ND * 128], BF16, name="wv_p", tag="wv_p")
            nc.scalar.dma_start(out=wk_p.rearrange("p (n j) -> p n j", j=128),
                                in_=wk.rearrange("(n p) j -> p n j", p=128))
            mt = wp.tile([128, 2], F32, name="mt", tag="mt")
            nc.scalar.dma_start(out=mt, in_=maskb[:, :])
            nc.scalar.dma_start(out=wq_p.rearrange("p (n j) -> p n j", j=128),
                                in_=wq.rearrange("(n p) j -> p n j", p=128))
            wk_t = [wk_p[:, d * 128:(d + 1) * 128] for d in range(ND)]
            wq_t = [wq_p[:, d * 128:(d + 1) * 128] for d in range(ND)]
            wv_t = [wv_p[:, d * 128:(d + 1) * 128] for d in range(ND)]
            # exp table preload: a 1-element exp so the ~2.7us ACT table
            # load happens during the projection lead-in, not mid-pipeline
            scr1 = wp.tile([1, 1], F32, name="scr1", tag="scr1")
            nc.scalar.activation(scr1, mt[0:1, 0:1], AF.Exp)

            # ---- persistent activations (chunked for dep granularity) ----
            def kcols(b):
                return nks[b] * 128

            def round_widths(total):
                w = []
                while total > 0:
                    w.append(min(512, total))
                    total -= w[-1]
                return w

            KTt = [[per.tile([128, w], F32R, name=f"KT{b}_{i}",
                             tag=f"KT{b}_{i}")
                    for i, w in enumerate(round_widths(kcols(b)))]
                   for b in range(2)]
            QTt = [[per.tile([128, 512], F32R, name=f"QT{b}_{r}",
                             tag=f"QT{b}_{r}") for r in range(NQS)]
                   for b in range(2)]
            Vn = [[per.tile([128, HLOC * 65], F32R, name=f"Vn{b}_{i}",
                            tag=f"Vn{b}_{i}") for i in range(nks[b])]
                  for b in range(2)]
            OcT = [[per.tile([128, 512], F32R, name=f"OcT{b}_{q}",
                             tag=f"OcT{b}_{q}") for q in range(NQS)]
                   for b in range(2)]

            def kt_slice(b, kc):
                """KT stationary slice [*, kc*128:(kc+1)*128] across tiles."""
                c0 = kc * 128
                ti, off = divmod(c0, 512)
                return KTt[b][ti][:, off:off + 128]

            def kq_round(nm, xdram, wt, OUT, b, r):
                """One 512-wide Q projection round for batch b."""
                c0 = r * 512
                xt = xp.tile([128, ND * 512], BF16, name=f"x{nm}{b}_{r}",
                             tag="xin")
                nc.sync.dma_start(
                    out=xt.rearrange("p (n s) -> p n s", n=ND),
                    in_=xdram[:, c0:c0 + 512]
                    .rearrange("(n p) s -> p n s", p=128))
                xts = xt.rearrange("p (n s) -> p n s", n=ND)
                pt = pp.tile([128, 512], F32, name=f"p{nm}{b}_{r}", tag="pp")
                for d in range(ND):
                    nc.tensor.matmul(
                        pt, wt[d], xts[:, d, :],
                        start=(d == 0), stop=(d == ND - 1))
                nc.vector.tensor_copy(OUT, pt)

            def k_rounds(b):
                """K^T projection for all nk_b k-chunks of batch b."""
                w = kcols(b)
                xkt = kvp.tile([128, ND * w], BF16, name=f"xk{b}", tag="xkv")
                xks = xkt.rearrange("p (n s) -> p n s", n=ND)
                xkd = xk[b].rearrange("(n p) s -> p n s", p=128)
                for half in range(2):
                    dsl = slice(half * (ND // 2), (half + 1) * (ND // 2))
                    nc.gpsimd.dma_start(out=xks[:, dsl, :],
                                        in_=xkd[:, dsl, :])
                for i, rw in enumerate(round_widths(w)):
                    c0 = i * 512
                    pt = pp.tile([128, rw], F32, name=f"pk{b}_{i}", tag="pp")
                    for d in range(ND):
                        nc.tensor.matmul(
                            pt, wk_t[d], xks[:, d, c0:c0 + rw],
                            start=(d == 0), stop=(d == ND - 1))
                    nc.vector.tensor_copy(KTt[b][i], pt)

            wv_loaded = [False]

            def v_round(b):
                """V natural-layout projection for all k-chunks of batch b."""
                if not wv_loaded[0]:
                    wv_loaded[0] = True
                    nc.gpsimd.dma_start(
                        out=wv_p.rearrange("p (n j) -> p n j", j=128),
                        in_=wv.rearrange("(n p) j -> p n j", p=128))
                w = kcols(b)
                xvt = kvp.tile([128, ND * w], BF16, name=f"xv{b}", tag="xkv")
                nc.gpsimd.dma_start(
                    out=xvt.rearrange("p (n s) -> p n s", n=ND),
                    in_=xv[b].rearrange("(n p) s -> p n s", p=128))
                xvs = xvt.rearrange("p (n s) -> p n s", n=ND)
                for kc in range(nks[b]):
                    pt = pp.tile([128, 128], F32, name=f"pv{b}_{kc}",
                                 tag="pp")
                    for d in range(ND):
                        nc.tensor.matmul(
                            pt, xvs[:, d, kc * 128:(kc + 1) * 128], wv_t[d],
                            start=(d == 0), stop=(d == ND - 1))
                    vt = Vn[b][kc]
                    vspl = vt.rearrange("p (h x) -> p h x", x=65)
                    nc.vector.memset(vspl[:, :, 64:65].bitcast(F32), 1.0)
                    nc.vector.tensor_copy(
                        vspl[:, :, 0:64],
                        pt.rearrange("p (h j) -> p h j", j=64))

            ncopy = [0]

            def spread_copy(dst, src, engines):
                # round-robin big PSUM->SBUF copies across the given engines
                eng = engines[ncopy[0] % len(engines)]
                ncopy[0] += 1
                if eng is nc.scalar:
                    eng.copy(dst, src)
                else:
                    eng.tensor_copy(dst, src)

            fills = []
            in_attn = [False]

            def pop_fill(n=1):
                for _ in range(n):
                    if fills:
                        fills.pop(0)()

            def flush_fills():
                while fills:
                    fills.pop(0)()

            def attention(b, h, qw):
                """One head, one 1024-wide q window of batch b.

                Scores run one kc ahead of AV so the in-order PE queue is
                never parked on an exp dependency; fill thunks (projection
                rounds / wo chunks) are drained between kc steps.
                """
                nk = nks[b]
                in_attn[0] = True
                hoff = h * 64
                pots = [po.tile([65, 512], F32, name=f"pot{b}_{h}_{qw}_{qh}",
                                tag="po") for qh in range(2)]
                ets = [None] * nk

                def scores_exp(kc):
                    pst = psc.tile([128, 1024], F32,
                                   name=f"pst{b}_{h}_{qw}_{kc}", tag="psc")
                    for qh in range(2):
                        nc.tensor.matmul(
                            pst[:, qh * 512:(qh + 1) * 512],
                            kt_slice(b, kc)[hoff:hoff + 64, :],
                            QTt[b][2 * qw + qh][hoff:hoff + 64, :],
                            start=True, stop=True)
                    et = ep.tile([128, 1024], F32R,
                                 name=f"et{b}_{h}_{qw}_{kc}", tag="et")
                    bias = mt[:, b:b + 1] if kc == nk - 1 else 0.0
                    nc.scalar.activation(et, pst, AF.Exp,
                                         bias=bias, scale=float(SCALE))
                    ets[kc] = et

                scores_exp(0)
                for kc in range(nk):
                    if kc + 1 < nk:
                        scores_exp(kc + 1)
                    for qh in range(2):
                        nc.tensor.matmul(
                            pots[qh],
                            Vn[b][kc][:, h * 65:h * 65 + 65],
                            ets[kc][:, qh * 512:(qh + 1) * 512],
                            start=(kc == 0), stop=(kc == nk - 1))
                    pop_fill(2 if len(fills) >= fill_slots[0] else 1)
                    fill_slots[0] = max(1, fill_slots[0] - 1)
                # 1/den can start as soon as the denominator rows are
                # staged; un copies (DVE+ACT) drain pots for Pool's muls
                # (Pool cannot read PSUM)
                dtmp = rbp.tile([1, 1024], F32, name=f"dt{b}{h}{qw}",
                                tag="tmp1")
                for qh in range(2):
                    nc.vector.tensor_copy(dtmp[:, qh * 512:(qh + 1) * 512],
                                          pots[qh][64:65, :])
                rrow = rbp.tile([1, 1024], F32, name=f"rr{b}{h}{qw}",
                                tag="tmp1")
                nc.vector.reciprocal(rrow, dtmp)
                rb = rbp.tile([64, 1024], F32, name=f"rb{b}{h}{qw}", tag="rb")
                nc.gpsimd.partition_broadcast(rb, rrow[0:1, :])
                uns = []
                for qh in range(2):
                    un = rbp.tile([64, 512], F32, name=f"un{b}{h}{qw}{qh}",
                                  tag="un")
                    if qh == 0:
                        nc.vector.tensor_copy(un, pots[qh][0:64, :])
                    else:
                        nc.scalar.copy(un, pots[qh][0:64, :])
                    uns.append(un)
                for qh in range(2):
                    nc.gpsimd.tensor_mul(
                        OcT[b][2 * qw + qh][hoff:hoff + 64, :],
                        uns[qh], rb[:, qh * 512:(qh + 1) * 512])
                in_attn[0] = False

            fill_slots = [0]

            nwo = [0]

            def wo_chunk(b, st, use_psc=False):
                """One 128-row output chunk of batch b's partial product."""
                qs, i = divmod(st, 4)
                ot = outp.tile([128, 1024], BF16, name=f"ot{b}_{st}",
                               tag="ot")
                if use_psc:
                    pt = psc.tile([128, 1024], F32, name=f"pw{b}_{st}",
                                  tag="psc")
                    for odh in range(2):
                        nc.tensor.matmul(
                            pt[:, odh * 512:(odh + 1) * 512],
                            OcT[b][qs][:, i * 128:(i + 1) * 128],
                            wo_t[:, odh * 512:(odh + 1) * 512],
                            start=True, stop=True)
                    s6 = nwo[0] % 6
                    if s6 in ((1, 4) if in_attn[0] else (1, 3, 5)):
                        nc.scalar.copy(ot, pt)
                    else:
                        nc.vector.tensor_copy(ot, pt)
                    nwo[0] += 1
                    eng = nc.sync if (st % 2 == 0) else nc.gpsimd
                    eng.dma_start(
                        out=out[b][st * 128:(st + 1) * 128, :], in_=ot)
                    return
                for odh in range(2):
                    pt = pp.tile([128, 512], F32, name=f"pw{b}_{st}_{odh}",
                                 tag="pp")
                    nc.tensor.matmul(
                        pt,
                        OcT[b][qs][:, i * 128:(i + 1) * 128],
                        wo_t[:, odh * 512:(odh + 1) * 512],
                        start=True, stop=True)
                    # exp saturates ACT inside attention windows: drain
                    # on DVE there, alternate DVE/ACT outside them
                    s6 = nwo[0] % 6
                    if s6 in ((1, 4) if in_attn[0] else (1, 3, 5)):
                        nc.scalar.copy(ot[:, odh * 512:(odh + 1) * 512], pt)
                    else:
                        nc.vector.tensor_copy(
                            ot[:, odh * 512:(odh + 1) * 512], pt)
                    nwo[0] += 1
                eng = nc.sync if (st % 2 == 0) else nc.gpsimd
                eng.dma_start(
                    out=out[b][st * 128:(st + 1) * 128, :], in_=ot)

            # ---- emission schedule ----
            k_rounds(0)
            kq_round("q", xq[0], wq_t, QTt[0][0], 0, 0)
            v_round(0)
            kq_round("q", xq[0], wq_t, QTt[0][1], 0, 1)

            wo_p = wp.tile([128, D], F32R, name="wo_p", tag="wo_p")
            wo_t = wo_p

            # b0 attention, PE fed by remaining projection thunks
            fills.extend([
                lambda: kq_round("q", xq[0], wq_t, QTt[0][2], 0, 2),
                lambda: kq_round("q", xq[0], wq_t, QTt[0][3], 0, 3),
                lambda: k_rounds(1),
                lambda: v_round(1),
                lambda: kq_round("q", xq[1], wq_t, QTt[1][0], 1, 0),
                lambda: kq_round("q", xq[1], wq_t, QTt[1][1], 1, 1),
                lambda: kq_round("q", xq[1], wq_t, QTt[1][2], 1, 2),
                lambda: kq_round("q", xq[1], wq_t, QTt[1][3], 1, 3),
            ])
            fill_slots[0] = 4 * nks[0]
            attention(0, 0, 0)
            attention(0, 1, 0)
            nc.sync.dma_start(out=wo_p, in_=wo[:, :])
            attention(0, 0, 1)
            attention(0, 1, 1)
            flush_fills()

            # b1 attention, PE fed by wo chunks; wo(1, qs) appended once its
            # OcT chunks' producers are emitted.
            fill_slots[0] = 4 * nks[1]
            for st in range(16):
                fills.append(lambda st=st: wo_chunk(0, st))
            attention(1, 0, 0)
            attention(1, 1, 0)
            for st in range(8):
                fills.append(lambda st=st: wo_chunk(1, st))
            attention(1, 0, 1)
            attention(1, 1, 1)
            flush_fills()
            for j, st in enumerate(range(8, 16)):
                wo_chunk(1, st, use_psc=(j % 2 == 0))
    nc.compile()
    return nc


def _get_nc(nk0, nk1):
    key = (nk0, nk1)
    if key not in _cached:
        _cached[key] = _build(nk0, nk1)
    return _cached[key]


def kernel(queries, keys, values, valid_lens, Wq, Wk, Wv, Wo, **kwargs):
    queries = np.asarray(queries, dtype=np.float32)
    keys = np.asarray(keys, dtype=np.float32)
    values = np.asarray(values, dtype=np.float32)
    Wq = np.asarray(Wq, dtype=np.float32)
    Wk = np.asarray(Wk, dtype=np.float32)
    Wv = np.asarray(Wv, dtype=np.float32)
    Wo = np.asarray(Wo, dtype=np.float32)
    vls = np.asarray(valid_lens).astype(np.int64)
    B = queries.shape[0]
    assert B == 2 and queries.shape[1:] == (S, D), \
        f"kernel compiled for (2, {S}, {D}), got {queries.shape}"

    nks = [int(min(16, max(1, -(-int(vls[b]) // 128)))) for b in range(B)]
    nc = _get_nc(nks[0], nks[1])

    bf16 = ml_dtypes.bfloat16
    xqs, xks, xvs, mks = [], [], [], []
    for b in range(B):
        vl = int(vls[b])
        nk = nks[b]
        qb = queries[b]
        if vl <= 0:
            # reference: fully-masked row -> softmax of constant -> uniform.
            # (cannot happen with this reference's randint(1, S+1) bounds)
            qb = np.zeros_like(qb)
            mk = np.zeros(128, np.float32)
        else:
            pos = (nk - 1) * 128 + np.arange(128)
            mk = np.where(pos < vl, 0.0, MASK_VALUE).astype(np.float32)
        mks.append(mk)
        xqs.append(np.ascontiguousarray(qb.T).astype(bf16))
        xks.append(np.ascontiguousarray(keys[b].T[:, :nk * 128]).astype(bf16))
        xvs.append(np.ascontiguousarray(values[b].T[:, :nk * 128]).astype(bf16))
    mkt = np.ascontiguousarray(np.stack(mks, axis=1))  # [128, 2]

    in_maps = []
    for c in range(8):
        sl = slice(c * 128, (c + 1) * 128)
        in_maps.append({
            "xq0": xqs[0], "xq1": xqs[1],
            "xk0": xks[0], "xk1": xks[1],
            "xv0": xvs[0], "xv1": xvs[1],
            "wq": np.ascontiguousarray(Wq[:, sl]).astype(bf16),
            "wk": np.ascontiguousarray(Wk[:, sl]).astype(bf16),
            "wv": np.ascontiguousarray(Wv[:, sl]).astype(bf16),
            "wo": np.ascontiguousarray(Wo[sl, :]),
            "maskb": mkt,
        })

    res = run_bass_kernel_spmd(nc, in_maps, core_ids=list(range(8)), **kwargs)
    global LAST_RESULTS
    LAST_RESULTS = res

    outp = np.zeros((B, S, D), np.float32)
    for b in range(B):
        acc = res.results[0][f"out{b}"].astype(np.float32)
        for c in range(1, 8):
            acc = acc + res.results[c][f"out{b}"].astype(np.float32)
        outp[b] = acc
    return outp


# revision 5
# speedup vs baseline: 2.3036x; 1.0133x over previous
"""Multi-head attention (16 heads, D=1024, B=2, S=2048) on 8 Trainium2 cores.

Sharding v2: head-wise tensor parallel — each core owns 2 heads (128 of the
1024 projection dims) and processes BOTH batches.  Per-core partial outputs
(full [2, 2048, 1024] shape through its 128 rows of Wo) are summed on host.

Key optimization vs v1: `valid_lens` is known at kernel-build time and masks
all scores at k >= vl to exp(-1e6) == 0 exactly, so k-chunks beyond
ceil(vl/128) contribute nothing to numerator or denominator.  The kernel is
compiled per (nk0, nk1) = ceil(vl/128) and never computes the masked
K/V projections, scores, exps, or AV products.  With vl=[288, 576] that cuts
attention work 4x and K/V projection work 3.2x, and head-wise sharding keeps
all 8 cores perfectly balanced (each sees both batches).

Device layout (per core, per batch b):
  X^T (feature-major, bf16) --(Wq/Wk stationary)--> QT/KT [j=128 dims, s]
  KT.T @ QT = scores^T [k, q] --exp(scale*x + mask)--> E [k, q]  (f32r)
  Vn natural [k, 2*65] (64 dims + ones col per head):
    Vn_h.T @ E accumulates attn-weighted V AND the softmax denominator
    (row 64) in one PSUM accumulation group.
  normalize: OcT[h*64:, q] = pots[0:64] * broadcast(1/pots[64])
  wo: OcT chunk [128, 128] stationary x Wo rows [128, 1024] -> out partial.

All attention matmuls run f32r at 1 cycle/row (free dim >= 256; the V/K-proj
tails with free 128 are bf16-input).  Outputs are written bf16 to halve the
output DMA; host accumulates the 8 partials in f32.
"""
import ml_dtypes
import numpy as np

import concourse.bacc as bacc
import concourse.mybir as mybir
import concourse.tile as tile
from concourse.bass_utils import run_bass_kernel_spmd

F32 = mybir.dt.float32
F32R = mybir.dt.float32r
BF16 = mybir.dt.bfloat16
AF = mybir.ActivationFunctionType

S = 2048          # sequence length
D = 1024          # model dim
HLOC = 2          # heads per core
HD = 64           # head dim
SCALE = 1.0 / np.sqrt(32.0)   # reference bug: d_k = B*H = 32
MASK_VALUE = -1.0e6

ND = 8            # d chunks of 128 (contraction for projections)
NQS = 4           # q chunks of 512 per batch (OcT/QT chunk granularity)

_cached = {}
LAST_RESULTS = None


def _build(nk0, nk1):
    nks = [nk0, nk1]
    nc = bacc.Bacc("TRN2", target_bir_lowering=False, debug=False,
                   num_swdge_queues=4)

    xq = [nc.dram_tensor(f"xq{b}", [D, S], BF16, kind="ExternalInput")
          for b in range(2)]
    xk = [nc.dram_tensor(f"xk{b}", [D, nks[b] * 128], BF16,
                         kind="ExternalInput") for b in range(2)]
    xv = [nc.dram_tensor(f"xv{b}", [D, nks[b] * 128], BF16,
                         kind="ExternalInput") for b in range(2)]
    wq = nc.dram_tensor("wq", [D, 128], BF16, kind="ExternalInput")
    wk = nc.dram_tensor("wk", [D, 128], BF16, kind="ExternalInput")
    wv = nc.dram_tensor("wv", [D, 128], BF16, kind="ExternalInput")
    wo = nc.dram_tensor("wo", [128, D], F32R, kind="ExternalInput")
    maskb = nc.dram_tensor("maskb", [128, 2], F32, kind="ExternalInput")
    out = [nc.dram_tensor(f"out{b}", [S, D], BF16, kind="ExternalOutput")
           for b in range(2)]

    with tile.TileContext(nc) as tc:
        with tc.tile_pool(name="wp", bufs=1) as wp, \
             tc.tile_pool(name="per", bufs=1) as per, \
             tc.tile_pool(name="xp", bufs=3) as xp, \
             tc.tile_pool(name="kvp", bufs=2) as kvp, \
             tc.tile_pool(name="ep", bufs=4) as ep, \
             tc.tile_pool(name="rbp", bufs=7) as rbp, \
             tc.tile_pool(name="outp", bufs=8) as outp, \
             tc.tile_pool(name="pp", bufs=2, space="PSUM") as pp, \
             tc.tile_pool(name="po", bufs=2, space="PSUM") as po, \
             tc.tile_pool(name="psc", bufs=2, space="PSUM") as psc:

            # ---- mask + packed projection weights ----
            # (wk first on SP and xk0 first on Pool: the K projection is the
            # head of the whole pipeline)
            wk_p = wp.tile([128, ND * 128], BF16, name="wk_p", tag="wk_p")
            wq_p = wp.tile([128, ND * 128], BF16, name="wq_p", tag="wq_p")
            wv_p = wp.tile([128, ND * 128], BF16, name="wv_p", tag="wv_p")
            nc.scalar.dma_start(out=wk_p.rearrange("p (n j) -> p n j", j=128),
                                in_=wk.rearrange("(n p) j -> p n j", p=128))
            mt = wp.tile([128, 2], F32, name="mt", tag="mt")
            nc.scalar.dma_start(out=mt, in_=maskb[:, :])
            nc.scalar.dma_start(out=wq_p.rearrange("p (n j) -> p n j", j=128),
                                in_=wq.rearrange("(n p) j -> p n j", p=128))
            wk_t = [wk_p[:, d * 128:(d + 1) * 128] for d in range(ND)]
            wq_t = [wq_p[:, d * 128:(d + 1) * 128] for d in range(ND)]
            wv_t = [wv_p[:, d * 128:(d + 1) * 128] for d in range(ND)]
            # exp table preload: a 1-element exp so the ~2.7us ACT table
            # load happens during the projection lead-in, not mid-pipeline
            scr1 = wp.tile([1, 1], F32, name="scr1", tag="scr1")
            nc.scalar.activation(scr1, mt[0:1, 0:1], AF.Exp)

            # ---- persistent activations (chunked for dep granularity) ----
            def kcols(b):
                return nks[b] * 128

            def round_widths(total):
                w = []
                while total > 0:
                    w.append(min(512, total))
                    total -= w[-1]
                return w

            KTt = [[per.tile([128, w], F32R, name=f"KT{b}_{i}",
                             tag=f"KT{b}_{i}")
                    for i, w in enumerate(round_widths(kcols(b)))]
                   for b in range(2)]
            QTt = [[per.tile([128, 512], F32R, name=f"QT{b}_{r}",
                             tag=f"QT{b}_{r}") for r in range(NQS)]
                   for b in range(2)]
            Vn = [[per.tile([128, HLOC * 65], F32R, name=f"Vn{b}_{i}",
                            tag=f"Vn{b}_{i}") for i in range(nks[b])]
                  for b in range(2)]
            OcT = [[per.tile([128, 512], F32R, name=f"OcT{b}_{q}",
                             tag=f"OcT{b}_{q}") for q in range(NQS)]
                   for b in range(2)]

            def kt_slice(b, kc):
                """KT stationary slice [*, kc*128:(kc+1)*128] across tiles."""
                c0 = kc * 128
                ti, off = divmod(c0, 512)
                return KTt[b][ti][:, off:off + 128]

            def kq_round(nm, xdram, wt, OUT, b, r, dma=None):
                """One 512-wide Q projection round for batch b."""
                c0 = r * 512
                xt = xp.tile([128, ND * 512], BF16, name=f"x{nm}{b}_{r}",
                             tag="xin")
                (dma or nc.sync).dma_start(
                    out=xt.rearrange("p (n s) -> p n s", n=ND),
                    in_=xdram[:, c0:c0 + 512]
                    .rearrange("(n p) s -> p n s", p=128))
                xts = xt.rearrange("p (n s) -> p n s", n=ND)
                pt = pp.tile([128, 512], F32, name=f"p{nm}{b}_{r}", tag="pp")
                for d in range(ND):
                    nc.tensor.matmul(
                        pt, wt[d], xts[:, d, :],
                        start=(d == 0), stop=(d == ND - 1))
                nc.vector.tensor_copy(OUT, pt)

            def k_rounds(b):
                """K^T projection for all nk_b k-chunks of batch b."""
                w = kcols(b)
                xkt = kvp.tile([128, ND * w], BF16, name=f"xk{b}", tag="xkv")
                xks = xkt.rearrange("p (n s) -> p n s", n=ND)
                xkd = xk[b].rearrange("(n p) s -> p n s", p=128)
                for half in range(2):
                    dsl = slice(half * (ND // 2), (half + 1) * (ND // 2))
                    nc.gpsimd.dma_start(out=xks[:, dsl, :],
                                        in_=xkd[:, dsl, :])
                for i, rw in enumerate(round_widths(w)):
                    c0 = i * 512
                    pt = pp.tile([128, rw], F32, name=f"pk{b}_{i}", tag="pp")
                    for d in range(ND):
                        nc.tensor.matmul(
                            pt, wk_t[d], xks[:, d, c0:c0 + rw],
                            start=(d == 0), stop=(d == ND - 1))
                    nc.vector.tensor_copy(KTt[b][i], pt)

            wv_loaded = [False]

            def v_round(b):
                """V natural-layout projection for all k-chunks of batch b."""
                if not wv_loaded[0]:
                    wv_loaded[0] = True
                    nc.gpsimd.dma_start(
                        out=wv_p.rearrange("p (n j) -> p n j", j=128),
                        in_=wv.rearrange("(n p) j -> p n j", p=128))
                w = kcols(b)
                xvt = kvp.tile([128, ND * w], BF16, name=f"xv{b}", tag="xkv")
                nc.gpsimd.dma_start(
                    out=xvt.rearrange("p (n s) -> p n s", n=ND),
                    in_=xv[b].rearrange("(n p) s -> p n s", p=128))
                xvs = xvt.rearrange("p (n s) -> p n s", n=ND)
                for kc in range(nks[b]):
                    pt = pp.tile([128, 128], F32, name=f"pv{b}_{kc}",
                                 tag="pp")
                    for d in range(ND):
                        nc.tensor.matmul(
                            pt, xvs[:, d, kc * 128:(kc + 1) * 128], wv_t[d],
                            start=(d == 0), stop=(d == ND - 1))
                    vt = Vn[b][kc]
                    vspl = vt.rearrange("p (h x) -> p h x", x=65)
                    nc.vector.memset(vspl[:, :, 64:65].bitcast(F32), 1.0)
                    nc.vector.tensor_copy(
                        vspl[:, :, 0:64],
                        pt.rearrange("p (h j) -> p h j", j=64))

            ncopy = [0]

            def spread_copy(dst, src, engines):
                # round-robin big PSUM->SBUF copies across the given engines
                eng = engines[ncopy[0] % len(engines)]
                ncopy[0] += 1
                if eng is nc.scalar:
                    eng.copy(dst, src)
                else:
                    eng.tensor_copy(dst, src)

            fills = []
            in_attn = [False]

            def pop_fill(n=1):
                for _ in range(n):
                    if fills:
                        fills.pop(0)()

            def flush_fills():
                while fills:
                    fills.pop(0)()

            def attention(b, h, qw, un_act=False):
                """One head, one 1024-wide q window of batch b.

                Scores run one kc ahead of AV so the in-order PE queue is
                never parked on an exp dependency; fill thunks (projection
                rounds / wo chunks) are drained between kc steps.
                """
                nk = nks[b]
                in_attn[0] = True
                hoff = h * 64
                pots = [po.tile([65, 512], F32, name=f"pot{b}_{h}_{qw}_{qh}",
                                tag="po") for qh in range(2)]
                ets = [None] * nk

                def scores_exp(kc):
                    pst = psc.tile([128, 1024], F32,
                                   name=f"pst{b}_{h}_{qw}_{kc}", tag="psc")
                    for qh in range(2):
                        nc.tensor.matmul(
                            pst[:, qh * 512:(qh + 1) * 512],
                            kt_slice(b, kc)[hoff:hoff + 64, :],
                            QTt[b][2 * qw + qh][hoff:hoff + 64, :],
                            start=True, stop=True)
                    et = ep.tile([128, 1024], F32R,
                                 name=f"et{b}_{h}_{qw}_{kc}", tag="et")
                    bias = mt[:, b:b + 1] if kc == nk - 1 else 0.0
                    nc.scalar.activation(et, pst, AF.Exp,
                                         bias=bias, scale=float(SCALE))
                    ets[kc] = et

                scores_exp(0)
                for kc in range(nk):
                    if kc + 1 < nk:
                        scores_exp(kc + 1)
                    for qh in range(2):
                        nc.tensor.matmul(
                            pots[qh],
                            Vn[b][kc][:, h * 65:h * 65 + 65],
                            ets[kc][:, qh * 512:(qh + 1) * 512],
                            start=(kc == 0), stop=(kc == nk - 1))
                    pop_fill(2 if len(fills) >= fill_slots[0] else 1)
                    fill_slots[0] = max(1, fill_slots[0] - 1)
                # 1/den can start as soon as the denominator rows are
                # staged; un copies (DVE+ACT) drain pots for Pool's muls
                # (Pool cannot read PSUM)
                dtmp = rbp.tile([1, 1024], F32, name=f"dt{b}{h}{qw}",
                                tag="tmp1")
                for qh in range(2):
                    nc.vector.tensor_copy(dtmp[:, qh * 512:(qh + 1) * 512],
                                          pots[qh][64:65, :])
                rrow = rbp.tile([1, 1024], F32, name=f"rr{b}{h}{qw}",
                                tag="tmp1")
                nc.vector.reciprocal(rrow, dtmp)
                rb = rbp.tile([64, 1024], F32, name=f"rb{b}{h}{qw}", tag="rb")
                nc.gpsimd.partition_broadcast(rb, rrow[0:1, :])
                uns = []
                for qh in range(2):
                    un = rbp.tile([64, 512], F32, name=f"un{b}{h}{qw}{qh}",
                                  tag="un")
                    if (qh == 1) or un_act:
                        nc.scalar.copy(un, pots[qh][0:64, :])
                    else:
                        nc.vector.tensor_copy(un, pots[qh][0:64, :])
                    uns.append(un)
                for qh in range(2):
                    nc.gpsimd.tensor_mul(
                        OcT[b][2 * qw + qh][hoff:hoff + 64, :],
                        uns[qh], rb[:, qh * 512:(qh + 1) * 512])
                in_attn[0] = False

            fill_slots = [0]

            nwo = [0]

            def wo_chunk(b, st, use_psc=False):
                """One 128-row output chunk of batch b's partial product."""
                qs, i = divmod(st, 4)
                ot = outp.tile([128, 1024], BF16, name=f"ot{b}_{st}",
                               tag="ot")
                if use_psc:
                    pt = psc.tile([128, 1024], F32, name=f"pw{b}_{st}",
                                  tag="psc")
                    for odh in range(2):
                        nc.tensor.matmul(
                            pt[:, odh * 512:(odh + 1) * 512],
                            OcT[b][qs][:, i * 128:(i + 1) * 128],
                            wo_t[:, odh * 512:(odh + 1) * 512],
                            start=True, stop=True)
                    s6 = nwo[0] % 6
                    if s6 in ((1, 4) if in_attn[0] else (1, 2, 4, 5)):
                        nc.scalar.copy(ot, pt)
                    else:
                        nc.vector.tensor_copy(ot, pt)
                    nwo[0] += 1
                    eng = nc.sync if (st % 2 == 0) else nc.gpsimd
                    eng.dma_start(
                        out=out[b][st * 128:(st + 1) * 128, :], in_=ot)
                    return
                for odh in range(2):
                    pt = pp.tile([128, 512], F32, name=f"pw{b}_{st}_{odh}",
                                 tag="pp")
                    nc.tensor.matmul(
                        pt,
                        OcT[b][qs][:, i * 128:(i + 1) * 128],
                        wo_t[:, odh * 512:(odh + 1) * 512],
                        start=True, stop=True)
                    # exp saturates ACT inside attention windows: drain
                    # on DVE there, alternate DVE/ACT outside them
                    s6 = nwo[0] % 6
                    if s6 in ((1, 4) if in_attn[0] else (1, 2, 4, 5)):
                        nc.scalar.copy(ot[:, odh * 512:(odh + 1) * 512], pt)
                    else:
                        nc.vector.tensor_copy(
                            ot[:, odh * 512:(odh + 1) * 512], pt)
                    nwo[0] += 1
                eng = nc.sync if (st % 2 == 0) else nc.gpsimd
                eng.dma_start(
                    out=out[b][st * 128:(st + 1) * 128, :], in_=ot)

            # ---- emission schedule ----
            k_rounds(0)
            kq_round("q", xq[0], wq_t, QTt[0][0], 0, 0)
            v_round(0)
            kq_round("q", xq[0], wq_t, QTt[0][1], 0, 1)

            wo_p = wp.tile([128, D], F32R, name="wo_p", tag="wo_p")
            wo_t = wo_p

            # b0 attention, PE fed by remaining projection thunks
            fills.extend([
                lambda: kq_round("q", xq[0], wq_t, QTt[0][2], 0, 2),
                lambda: kq_round("q", xq[0], wq_t, QTt[0][3], 0, 3),
                lambda: k_rounds(1),
                lambda: v_round(1),
                lambda: kq_round("q", xq[1], wq_t, QTt[1][0], 1, 0),
                lambda: kq_round("q", xq[1], wq_t, QTt[1][1], 1, 1, nc.gpsimd),
                lambda: kq_round("q", xq[1], wq_t, QTt[1][2], 1, 2),
                lambda: kq_round("q", xq[1], wq_t, QTt[1][3], 1, 3, nc.gpsimd),
            ])
            fill_slots[0] = 4 * nks[0]
            attention(0, 0, 0)
            attention(0, 1, 0)
            nc.sync.dma_start(out=wo_p, in_=wo[:, :])
            attention(0, 0, 1)
            attention(0, 1, 1)
            flush_fills()

            # b1 attention, PE fed by wo chunks; wo(1, qs) appended once its
            # OcT chunks' producers are emitted.
            fill_slots[0] = 4 * nks[1]
            for st in range(16):
                fills.append(lambda st=st: wo_chunk(0, st))
            attention(1, 0, 0)
            attention(1, 1, 0)
            for st in range(8):
                fills.append(lambda st=st: wo_chunk(1, st))
            attention(1, 0, 1, un_act=True)
            attention(1, 1, 1, un_act=True)
            flush_fills()
            for j, st in enumerate(range(8, 16)):
                wo_chunk(1, st, use_psc=(j % 2 == 0))
    nc.compile()
    return nc


def _get_nc(nk0, nk1):
    key = (nk0, nk1)
    if key not in _cached:
        _cached[key] = _build(nk0, nk1)
    return _cached[key]


def kernel(queries, keys, values, valid_lens, Wq, Wk, Wv, Wo, **kwargs):
    queries = np.asarray(queries, dtype=np.float32)
    keys = np.asarray(keys, dtype=np.float32)
    values = np.asarray(values, dtype=np.float32)
    Wq = np.asarray(Wq, dtype=np.float32)
    Wk = np.asarray(Wk, dtype=np.float32)
    Wv = np.asarray(Wv, dtype=np.float32)
    Wo = np.asarray(Wo, dtype=np.float32)
    vls = np.asarray(valid_lens).astype(np.int64)
    B = queries.shape[0]
    assert B == 2 and queries.shape[1:] == (S, D), \
        f"kernel compiled for (2, {S}, {D}), got {queries.shape}"

    nks = [int(min(16, max(1, -(-int(vls[b]) // 128)))) for b in range(B)]
    nc = _get_nc(nks[0], nks[1])

    bf16 = ml_dtypes.bfloat16
    xqs, xks, xvs, mks = [], [], [], []
    for b in range(B):
        vl = int(vls[b])
        nk = nks[b]
        qb = queries[b]
        if vl <= 0:
            # reference: fully-masked row -> softmax of constant -> uniform.
            # (cannot happen with this reference's randint(1, S+1) bounds)
            qb = np.zeros_like(qb)
            mk = np.zeros(128, np.float32)
        else:
            pos = (nk - 1) * 128 + np.arange(128)
            mk = np.where(pos < vl, 0.0, MASK_VALUE).astype(np.float32)
        mks.append(mk)
        xqs.append(np.ascontiguousarray(qb.T).astype(bf16))
        xks.append(np.ascontiguousarray(keys[b].T[:, :nk * 128]).astype(bf16))
        xvs.append(np.ascontiguousarray(values[b].T[:, :nk * 128]).astype(bf16))
    mkt = np.ascontiguousarray(np.stack(mks, axis=1))  # [128, 2]

    in_maps = []
    for c in range(8):
        sl = slice(c * 128, (c + 1) * 128)
        in_maps.append({
            "xq0": xqs[0], "xq1": xqs[1],
            "xk0": xks[0], "xk1": xks[1],
            "xv0": xvs[0], "xv1": xvs[1],
            "wq": np.ascontiguousarray(Wq[:, sl]).astype(bf16),
            "wk": np.ascontiguousarray(Wk[:, sl]).astype(bf16),
            "wv": np.ascontiguousarray(Wv[:, sl]).astype(bf16),
            "wo": np.ascontiguousarray(Wo[sl, :]),
            "maskb": mkt,
        })

    res = run_bass_kernel_spmd(nc, in_maps, core_ids=list(range(8)), **kwargs)
    global LAST_RESULTS
    LAST_RESULTS = res

    outp = np.zeros((B, S, D), np.float32)
    for b in range(B):
        acc = res.results[0][f"out{b}"].astype(np.float32)
        for c in range(1, 8):
            acc = acc + res.results[c][f"out{b}"].astype(np.float32)
        outp[b] = acc
    return outp


# revision 6
# speedup vs baseline: 2.3141x; 1.0046x over previous
"""Multi-head attention (16 heads, D=1024, B=2, S=2048) on 8 Trainium2 cores.

Sharding v2: head-wise tensor parallel — each core owns 2 heads (128 of the
1024 projection dims) and processes BOTH batches.  Per-core partial outputs
(full [2, 2048, 1024] shape through its 128 rows of Wo) are summed on host.

Key optimization vs v1: `valid_lens` is known at kernel-build time and masks
all scores at k >= vl to exp(-1e6) == 0 exactly, so k-chunks beyond
ceil(vl/128) contribute nothing to numerator or denominator.  The kernel is
compiled per (nk0, nk1) = ceil(vl/128) and never computes the masked
K/V projections, scores, exps, or AV products.  With vl=[288, 576] that cuts
attention work 4x and K/V projection work 3.2x, and head-wise sharding keeps
all 8 cores perfectly balanced (each sees both batches).

Device layout (per core, per batch b):
  X^T (feature-major, bf16) --(Wq/Wk stationary)--> QT/KT [j=128 dims, s]
  KT.T @ QT = scores^T [k, q] --exp(scale*x + mask)--> E [k, q]  (f32r)
  Vn natural [k, 2*65] (64 dims + ones col per head):
    Vn_h.T @ E accumulates attn-weighted V AND the softmax denominator
    (row 64) in one PSUM accumulation group.
  normalize: OcT[h*64:, q] = pots[0:64] * broadcast(1/pots[64])
  wo: OcT chunk [128, 128] stationary x Wo rows [128, 1024] -> out partial.

All attention matmuls run f32r at 1 cycle/row (free dim >= 256; the V/K-proj
tails with free 128 are bf16-input).  Outputs are written bf16 to halve the
output DMA; host accumulates the 8 partials in f32.
"""
import ml_dtypes
import numpy as np

import concourse.bacc as bacc
import concourse.mybir as mybir
import concourse.tile as tile
from concourse.bass_utils import run_bass_kernel_spmd

F32 = mybir.dt.float32
F32R = mybir.dt.float32r
BF16 = mybir.dt.bfloat16
AF = mybir.ActivationFunctionType

S = 2048          # sequence length
D = 1024          # model dim
HLOC = 2          # heads per core
HD = 64           # head dim
SCALE = 1.0 / np.sqrt(32.0)   # reference bug: d_k = B*H = 32
MASK_VALUE = -1.0e6

ND = 8            # d chunks of 128 (contraction for projections)
NQS = 4           # q chunks of 512 per batch (OcT/QT chunk granularity)

_cached = {}
LAST_RESULTS = None


def _build(nk0, nk1):
    nks = [nk0, nk1]
    nc = bacc.Bacc("TRN2", target_bir_lowering=False, debug=False,
                   num_swdge_queues=4)

    xq = [nc.dram_tensor(f"xq{b}", [D, S], BF16, kind="ExternalInput")
          for b in range(2)]
    xk = [nc.dram_tensor(f"xk{b}", [D, nks[b] * 128], BF16,
                         kind="ExternalInput") for b in range(2)]
    xv = [nc.dram_tensor(f"xv{b}", [D, nks[b] * 128], BF16,
                         kind="ExternalInput") for b in range(2)]
    wq = nc.dram_tensor("wq", [D, 128], BF16, kind="ExternalInput")
    wk = nc.dram_tensor("wk", [D, 128], BF16, kind="ExternalInput")
    wv = nc.dram_tensor("wv", [D, 128], BF16, kind="ExternalInput")
    wo = nc.dram_tensor("wo", [128, D], F32R, kind="ExternalInput")
    maskb = nc.dram_tensor("maskb", [128, 2], F32, kind="ExternalInput")
    out = [nc.dram_tensor(f"out{b}", [S, D], BF16, kind="ExternalOutput")
           for b in range(2)]

    with tile.TileContext(nc) as tc:
        with tc.tile_pool(name="wp", bufs=1) as wp, \
             tc.tile_pool(name="per", bufs=1) as per, \
             tc.tile_pool(name="xp", bufs=3) as xp, \
             tc.tile_pool(name="kvp", bufs=2) as kvp, \
             tc.tile_pool(name="ep", bufs=4) as ep, \
             tc.tile_pool(name="rbp", bufs=7) as rbp, \
             tc.tile_pool(name="outp", bufs=8) as outp, \
             tc.tile_pool(name="pp", bufs=2, space="PSUM") as pp, \
             tc.tile_pool(name="po", bufs=2, space="PSUM") as po, \
             tc.tile_pool(name="psc", bufs=2, space="PSUM") as psc:

            # ---- mask + packed projection weights ----
            # (wk first on SP and xk0 first on Pool: the K projection is the
            # head of the whole pipeline)
            wk_p = wp.tile([128, ND * 128], BF16, name="wk_p", tag="wk_p")
            wq_p = wp.tile([128, ND * 128], BF16, name="wq_p", tag="wq_p")
            wv_p = wp.tile([128, ND * 128], BF16, name="wv_p", tag="wv_p")
            nc.scalar.dma_start(out=wk_p.rearrange("p (n j) -> p n j", j=128),
                                in_=wk.rearrange("(n p) j -> p n j", p=128))
            mt = wp.tile([128, 2], F32, name="mt", tag="mt")
            nc.scalar.dma_start(out=mt, in_=maskb[:, :])
            nc.scalar.dma_start(out=wq_p.rearrange("p (n j) -> p n j", j=128),
                                in_=wq.rearrange("(n p) j -> p n j", p=128))
            wk_t = [wk_p[:, d * 128:(d + 1) * 128] for d in range(ND)]
            wq_t = [wq_p[:, d * 128:(d + 1) * 128] for d in range(ND)]
            wv_t = [wv_p[:, d * 128:(d + 1) * 128] for d in range(ND)]
            # exp table preload: a 1-element exp so the ~2.7us ACT table
            # load happens during the projection lead-in, not mid-pipeline
            scr1 = wp.tile([1, 1], F32, name="scr1", tag="scr1")
            nc.scalar.activation(scr1, mt[0:1, 0:1], AF.Exp)

            # ---- persistent activations (chunked for dep granularity) ----
            def kcols(b):
                return nks[b] * 128

            def round_widths(total):
                w = []
                while total > 0:
                    w.append(min(512, total))
                    total -= w[-1]
                return w

            KTt = [[per.tile([128, w], F32R, name=f"KT{b}_{i}",
                             tag=f"KT{b}_{i}")
                    for i, w in enumerate(round_widths(kcols(b)))]
                   for b in range(2)]
            QTt = [[per.tile([128, 512], F32R, name=f"QT{b}_{r}",
                             tag=f"QT{b}_{r}") for r in range(NQS)]
                   for b in range(2)]
            Vn = [[per.tile([128, HLOC * 65], F32R, name=f"Vn{b}_{i}",
                            tag=f"Vn{b}_{i}") for i in range(nks[b])]
                  for b in range(2)]
            OcT = [[per.tile([128, 512], F32R, name=f"OcT{b}_{q}",
                             tag=f"OcT{b}_{q}") for q in range(NQS)]
                   for b in range(2)]

            def kt_slice(b, kc):
                """KT stationary slice [*, kc*128:(kc+1)*128] across tiles."""
                c0 = kc * 128
                ti, off = divmod(c0, 512)
                return KTt[b][ti][:, off:off + 128]

            def kq_round(nm, xdram, wt, OUT, b, r, dma=None):
                """One 512-wide Q projection round for batch b."""
                c0 = r * 512
                xt = xp.tile([128, ND * 512], BF16, name=f"x{nm}{b}_{r}",
                             tag="xin")
                xts = xt.rearrange("p (n s) -> p n s", n=ND)
                xdr = xdram[:, c0:c0 + 512].rearrange("(n p) s -> p n s",
                                                      p=128)
                for half in range(2):
                    dsl = slice(half * (ND // 2), (half + 1) * (ND // 2))
                    (dma or nc.sync).dma_start(out=xts[:, dsl, :],
                                               in_=xdr[:, dsl, :])
                pt = pp.tile([128, 512], F32, name=f"p{nm}{b}_{r}", tag="pp")
                for d in range(ND):
                    nc.tensor.matmul(
                        pt, wt[d], xts[:, d, :],
                        start=(d == 0), stop=(d == ND - 1))
                nc.vector.tensor_copy(OUT, pt)

            def k_rounds(b):
                """K^T projection for all nk_b k-chunks of batch b."""
                w = kcols(b)
                xkt = kvp.tile([128, ND * w], BF16, name=f"xk{b}", tag="xkv")
                xks = xkt.rearrange("p (n s) -> p n s", n=ND)
                xkd = xk[b].rearrange("(n p) s -> p n s", p=128)
                for half in range(2):
                    dsl = slice(half * (ND // 2), (half + 1) * (ND // 2))
                    nc.gpsimd.dma_start(out=xks[:, dsl, :],
                                        in_=xkd[:, dsl, :])
                for i, rw in enumerate(round_widths(w)):
                    c0 = i * 512
                    pt = pp.tile([128, rw], F32, name=f"pk{b}_{i}", tag="pp")
                    for d in range(ND):
                        nc.tensor.matmul(
                            pt, wk_t[d], xks[:, d, c0:c0 + rw],
                            start=(d == 0), stop=(d == ND - 1))
                    nc.vector.tensor_copy(KTt[b][i], pt)

            wv_loaded = [False]

            def v_round(b):
                """V natural-layout projection for all k-chunks of batch b."""
                if not wv_loaded[0]:
                    wv_loaded[0] = True
                    nc.gpsimd.dma_start(
                        out=wv_p.rearrange("p (n j) -> p n j", j=128),
                        in_=wv.rearrange("(n p) j -> p n j", p=128))
                w = kcols(b)
                xvt = kvp.tile([128, ND * w], BF16, name=f"xv{b}", tag="xkv")
                nc.gpsimd.dma_start(
                    out=xvt.rearrange("p (n s) -> p n s", n=ND),
                    in_=xv[b].rearrange("(n p) s -> p n s", p=128))
                xvs = xvt.rearrange("p (n s) -> p n s", n=ND)
                for kc in range(nks[b]):
                    pt = pp.tile([128, 128], F32, name=f"pv{b}_{kc}",
                                 tag="pp")
                    for d in range(ND):
                        nc.tensor.matmul(
                            pt, xvs[:, d, kc * 128:(kc + 1) * 128], wv_t[d],
                            start=(d == 0), stop=(d == ND - 1))
                    vt = Vn[b][kc]
                    vspl = vt.rearrange("p (h x) -> p h x", x=65)
                    nc.vector.memset(vspl[:, :, 64:65].bitcast(F32), 1.0)
                    nc.vector.tensor_copy(
                        vspl[:, :, 0:64],
                        pt.rearrange("p (h j) -> p h j", j=64))

            ncopy = [0]

            def spread_copy(dst, src, engines):
                # round-robin big PSUM->SBUF copies across the given engines
                eng = engines[ncopy[0] % len(engines)]
                ncopy[0] += 1
                if eng is nc.scalar:
                    eng.copy(dst, src)
                else:
                    eng.tensor_copy(dst, src)

            fills = []
            in_attn = [False]

            def pop_fill(n=1):
                for _ in range(n):
                    if fills:
                        fills.pop(0)()

            def flush_fills():
                while fills:
                    fills.pop(0)()

            def attention(b, h, qw, un_act=False):
                """One head, one 1024-wide q window of batch b.

                Scores run one kc ahead of AV so the in-order PE queue is
                never parked on an exp dependency; fill thunks (projection
                rounds / wo chunks) are drained between kc steps.
                """
                nk = nks[b]
                in_attn[0] = True
                hoff = h * 64
                pots = [po.tile([65, 512], F32, name=f"pot{b}_{h}_{qw}_{qh}",
                                tag="po") for qh in range(2)]
                ets = [None] * nk

                def scores_exp(kc):
                    pst = psc.tile([128, 1024], F32,
                                   name=f"pst{b}_{h}_{qw}_{kc}", tag="psc")
                    for qh in range(2):
                        nc.tensor.matmul(
                            pst[:, qh * 512:(qh + 1) * 512],
                            kt_slice(b, kc)[hoff:hoff + 64, :],
                            QTt[b][2 * qw + qh][hoff:hoff + 64, :],
                            start=True, stop=True)
                    et = ep.tile([128, 1024], F32R,
                                 name=f"et{b}_{h}_{qw}_{kc}", tag="et")
                    bias = mt[:, b:b + 1] if kc == nk - 1 else 0.0
                    nc.scalar.activation(et, pst, AF.Exp,
                                         bias=bias, scale=float(SCALE))
                    ets[kc] = et

                scores_exp(0)
                for kc in range(nk):
                    if kc + 1 < nk:
                        scores_exp(kc + 1)
                    for qh in range(2):
                        nc.tensor.matmul(
                            pots[qh],
                            Vn[b][kc][:, h * 65:h * 65 + 65],
                            ets[kc][:, qh * 512:(qh + 1) * 512],
                            start=(kc == 0), stop=(kc == nk - 1))
                    pop_fill(2 if len(fills) >= fill_slots[0] else 1)
                    fill_slots[0] = max(1, fill_slots[0] - 1)
                # 1/den can start as soon as the denominator rows are
                # staged; un copies (DVE+ACT) drain pots for Pool's muls
                # (Pool cannot read PSUM)
                dtmp = rbp.tile([1, 1024], F32, name=f"dt{b}{h}{qw}",
                                tag="tmp1")
                for qh in range(2):
                    nc.vector.tensor_copy(dtmp[:, qh * 512:(qh + 1) * 512],
                                          pots[qh][64:65, :])
                rrow = rbp.tile([1, 1024], F32, name=f"rr{b}{h}{qw}",
                                tag="tmp1")
                nc.vector.reciprocal(rrow, dtmp)
                rb = rbp.tile([64, 1024], F32, name=f"rb{b}{h}{qw}", tag="rb")
                nc.gpsimd.partition_broadcast(rb, rrow[0:1, :])
                uns = []
                for qh in range(2):
                    un = rbp.tile([64, 512], F32, name=f"un{b}{h}{qw}{qh}",
                                  tag="un")
                    if (qh == 1) or un_act:
                        nc.scalar.copy(un, pots[qh][0:64, :])
                    else:
                        nc.vector.tensor_copy(un, pots[qh][0:64, :])
                    uns.append(un)
                for qh in range(2):
                    nc.gpsimd.tensor_mul(
                        OcT[b][2 * qw + qh][hoff:hoff + 64, :],
                        uns[qh], rb[:, qh * 512:(qh + 1) * 512])
                in_attn[0] = False

            fill_slots = [0]

            nwo = [0]

            def wo_chunk(b, st, use_psc=False):
                """One 128-row output chunk of batch b's partial product."""
                qs, i = divmod(st, 4)
                ot = outp.tile([128, 1024], BF16, name=f"ot{b}_{st}",
                               tag="ot")
                if use_psc:
                    pt = psc.tile([128, 1024], F32, name=f"pw{b}_{st}",
                                  tag="psc")
                    for odh in range(2):
                        nc.tensor.matmul(
                            pt[:, odh * 512:(odh + 1) * 512],
                            OcT[b][qs][:, i * 128:(i + 1) * 128],
                            wo_t[:, odh * 512:(odh + 1) * 512],
                            start=True, stop=True)
                    s6 = nwo[0] % 6
                    if s6 in ((1, 4) if in_attn[0] else (1, 2, 4, 5)):
                        nc.scalar.copy(ot, pt)
                    else:
                        nc.vector.tensor_copy(ot, pt)
                    nwo[0] += 1
                    eng = nc.sync if (st % 2 == 0) else nc.gpsimd
                    eng.dma_start(
                        out=out[b][st * 128:(st + 1) * 128, :], in_=ot)
                    return
                for odh in range(2):
                    pt = pp.tile([128, 512], F32, name=f"pw{b}_{st}_{odh}",
                                 tag="pp")
                    nc.tensor.matmul(
                        pt,
                        OcT[b][qs][:, i * 128:(i + 1) * 128],
                        wo_t[:, odh * 512:(odh + 1) * 512],
                        start=True, stop=True)
                    # exp saturates ACT inside attention windows: drain
                    # on DVE there, alternate DVE/ACT outside them
                    s6 = nwo[0] % 6
                    if s6 in ((1, 4) if in_attn[0] else (1, 2, 4, 5)):
                        nc.scalar.copy(ot[:, odh * 512:(odh + 1) * 512], pt)
                    else:
                        nc.vector.tensor_copy(
                            ot[:, odh * 512:(odh + 1) * 512], pt)
                    nwo[0] += 1
                eng = nc.sync if (st % 2 == 0) else nc.gpsimd
                eng.dma_start(
                    out=out[b][st * 128:(st + 1) * 128, :], in_=ot)

            # ---- emission schedule ----
            k_rounds(0)
            kq_round("q", xq[0], wq_t, QTt[0][0], 0, 0)
            v_round(0)
            kq_round("q", xq[0], wq_t, QTt[0][1], 0, 1)

            wo_p = wp.tile([128, D], F32R, name="wo_p", tag="wo_p")
            wo_t = wo_p

            # b0 attention, PE fed by remaining projection thunks
            fills.extend([
                lambda: kq_round("q", xq[0], wq_t, QTt[0][2], 0, 2),
                lambda: kq_round("q", xq[0], wq_t, QTt[0][3], 0, 3),
                lambda: k_rounds(1),
                lambda: v_round(1),
                lambda: kq_round("q", xq[1], wq_t, QTt[1][0], 1, 0),
                lambda: kq_round("q", xq[1], wq_t, QTt[1][1], 1, 1, nc.gpsimd),
                lambda: kq_round("q", xq[1], wq_t, QTt[1][2], 1, 2),
                lambda: kq_round("q", xq[1], wq_t, QTt[1][3], 1, 3, nc.gpsimd),
            ])
            fill_slots[0] = 4 * nks[0]
            attention(0, 0, 0)
            attention(0, 1, 0)
            nc.sync.dma_start(out=wo_p, in_=wo[:, :])
            attention(0, 0, 1)
            attention(0, 1, 1)
            flush_fills()

            # b1 attention, PE fed by wo chunks; wo(1, qs) appended once its
            # OcT chunks' producers are emitted.
            fill_slots[0] = 4 * nks[1]
            for st in range(16):
                fills.append(lambda st=st: wo_chunk(0, st))
            attention(1, 0, 0)
            attention(1, 1, 0)
            for st in range(8):
                fills.append(lambda st=st: wo_chunk(1, st))
            attention(1, 0, 1, un_act=True)
            attention(1, 1, 1, un_act=True)
            flush_fills()
            for j, st in enumerate(range(8, 16)):
                wo_chunk(1, st, use_psc=(j % 2 == 0))
    nc.compile()
    return nc


def _get_nc(nk0, nk1):
    key = (nk0, nk1)
    if key not in _cached:
        _cached[key] = _build(nk0, nk1)
    return _cached[key]


def kernel(queries, keys, values, valid_lens, Wq, Wk, Wv, Wo, **kwargs):
    queries = np.asarray(queries, dtype=np.float32)
    keys = np.asarray(keys, dtype=np.float32)
    values = np.asarray(values, dtype=np.float32)
    Wq = np.asarray(Wq, dtype=np.float32)
    Wk = np.asarray(Wk, dtype=np.float32)
    Wv = np.asarray(Wv, dtype=np.float32)
    Wo = np.asarray(Wo, dtype=np.float32)
    vls = np.asarray(valid_lens).astype(np.int64)
    B = queries.shape[0]
    assert B == 2 and queries.shape[1:] == (S, D), \
        f"kernel compiled for (2, {S}, {D}), got {queries.shape}"

    nks = [int(min(16, max(1, -(-int(vls[b]) // 128)))) for b in range(B)]
    nc = _get_nc(nks[0], nks[1])

    bf16 = ml_dtypes.bfloat16
    xqs, xks, xvs, mks = [], [], [], []
    for b in range(B):
        vl = int(vls[b])
        nk = nks[b]
        qb = queries[b]
        if vl <= 0:
            # reference: fully-masked row -> softmax of constant -> uniform.
            # (cannot happen with this reference's randint(1, S+1) bounds)
            qb = np.zeros_like(qb)
            mk = np.zeros(128, np.float32)
        else:
            pos = (nk - 1) * 128 + np.arange(128)
            mk = np.where(pos < vl, 0.0, MASK_VALUE).astype(np.float32)
        mks.append(mk)
        xqs.append(np.ascontiguousarray(qb.T).astype(bf16))
        xks.append(np.ascontiguousarray(keys[b].T[:, :nk * 128]).astype(bf16))
        xvs.append(np.ascontiguousarray(values[b].T[:, :nk * 128]).astype(bf16))
    mkt = np.ascontiguousarray(np.stack(mks, axis=1))  # [128, 2]

    in_maps = []
    for c in range(8):
        sl = slice(c * 128, (c + 1) * 128)
        in_maps.append({
            "xq0": xqs[0], "xq1": xqs[1],
            "xk0": xks[0], "xk1": xks[1],
            "xv0": xvs[0], "xv1": xvs[1],
            "wq": np.ascontiguousarray(Wq[:, sl]).astype(bf16),
            "wk": np.ascontiguousarray(Wk[:, sl]).astype(bf16),
            "wv": np.ascontiguousarray(Wv[:, sl]).astype(bf16),
            "wo": np.ascontiguousarray(Wo[sl, :]),
            "maskb": mkt,
        })

    res = run_bass_kernel_spmd(nc, in_maps, core_ids=list(range(8)), **kwargs)
    global LAST_RESULTS
    LAST_RESULTS = res

    outp = np.zeros((B, S, D), np.float32)
    for b in range(B):
        acc = res.results[0][f"out{b}"].astype(np.float32)
        for c in range(1, 8):
            acc = acc + res.results[c][f"out{b}"].astype(np.float32)
        outp[b] = acc
    return outp


# revision 7
# speedup vs baseline: 2.3703x; 1.0243x over previous
"""Multi-head attention (16 heads, D=1024, B=2, S=2048) on 8 Trainium2 cores.

Sharding v2: head-wise tensor parallel — each core owns 2 heads (128 of the
1024 projection dims) and processes BOTH batches.  Per-core partial outputs
(full [2, 2048, 1024] shape through its 128 rows of Wo) are summed on host.

Key optimization vs v1: `valid_lens` is known at kernel-build time and masks
all scores at k >= vl to exp(-1e6) == 0 exactly, so k-chunks beyond
ceil(vl/128) contribute nothing to numerator or denominator.  The kernel is
compiled per (nk0, nk1) = ceil(vl/128) and never computes the masked
K/V projections, scores, exps, or AV products.  With vl=[288, 576] that cuts
attention work 4x and K/V projection work 3.2x, and head-wise sharding keeps
all 8 cores perfectly balanced (each sees both batches).

Device layout (per core, per batch b):
  X^T (feature-major, bf16) --(Wq/Wk stationary)--> QT/KT [j=128 dims, s]
  KT.T @ QT = scores^T [k, q] --exp(scale*x + mask)--> E [k, q]  (f32r)
  Vn natural [k, 2*65] (64 dims + ones col per head):
    Vn_h.T @ E accumulates attn-weighted V AND the softmax denominator
    (row 64) in one PSUM accumulation group.
  normalize: OcT[h*64:, q] = pots[0:64] * broadcast(1/pots[64])
  wo: OcT chunk [128, 128] stationary x Wo rows [128, 1024] -> out partial.

All attention matmuls run f32r at 1 cycle/row (free dim >= 256; the V/K-proj
tails with free 128 are bf16-input).  Outputs are written bf16 to halve the
output DMA; host accumulates the 8 partials in f32.
"""
import ml_dtypes
import numpy as np

import concourse.bacc as bacc
import concourse.mybir as mybir
import concourse.tile as tile
from concourse.bass_utils import run_bass_kernel_spmd

F32 = mybir.dt.float32
F32R = mybir.dt.float32r
BF16 = mybir.dt.bfloat16
AF = mybir.ActivationFunctionType

S = 2048          # sequence length
D = 1024          # model dim
HLOC = 2          # heads per core
HD = 64           # head dim
SCALE = 1.0 / np.sqrt(32.0)   # reference bug: d_k = B*H = 32
MASK_VALUE = -1.0e6

ND = 8            # d chunks of 128 (contraction for projections)
NQS = 4           # q chunks of 512 per batch (OcT/QT chunk granularity)

_cached = {}
LAST_RESULTS = None


def _build(nk0, nk1):
    nks = [nk0, nk1]
    nc = bacc.Bacc("TRN2", target_bir_lowering=False, debug=False,
                   num_swdge_queues=4)

    xq = [nc.dram_tensor(f"xq{b}", [D, S], BF16, kind="ExternalInput")
          for b in range(2)]
    xk = [nc.dram_tensor(f"xk{b}", [D, nks[b] * 128], BF16,
                         kind="ExternalInput") for b in range(2)]
    xv = [nc.dram_tensor(f"xv{b}", [D, nks[b] * 128], BF16,
                         kind="ExternalInput") for b in range(2)]
    wq = nc.dram_tensor("wq", [D, 128], BF16, kind="ExternalInput")
    wk = nc.dram_tensor("wk", [D, 128], BF16, kind="ExternalInput")
    wv = nc.dram_tensor("wv", [D, 128], BF16, kind="ExternalInput")
    wo = nc.dram_tensor("wo", [128, D], F32R, kind="ExternalInput")
    maskb = nc.dram_tensor("maskb", [128, 2], F32, kind="ExternalInput")
    out = [nc.dram_tensor(f"out{b}", [S, D], BF16, kind="ExternalOutput")
           for b in range(2)]

    with tile.TileContext(nc) as tc:
        with tc.tile_pool(name="wp", bufs=1) as wp, \
             tc.tile_pool(name="per", bufs=1) as per, \
             tc.tile_pool(name="xp", bufs=3) as xp, \
             tc.tile_pool(name="kvp", bufs=2) as kvp, \
             tc.tile_pool(name="ep", bufs=4) as ep, \
             tc.tile_pool(name="rbp", bufs=7) as rbp, \
             tc.tile_pool(name="outp", bufs=8) as outp, \
             tc.tile_pool(name="pp", bufs=2, space="PSUM") as pp, \
             tc.tile_pool(name="po", bufs=2, space="PSUM") as po, \
             tc.tile_pool(name="psc", bufs=2, space="PSUM") as psc:

            # ---- mask + packed projection weights ----
            # (wk first on SP and xk0 first on Pool: the K projection is the
            # head of the whole pipeline)
            wk_p = wp.tile([128, ND * 128], BF16, name="wk_p", tag="wk_p")
            wq_p = wp.tile([128, ND * 128], BF16, name="wq_p", tag="wq_p")
            wv_p = wp.tile([128, ND * 128], BF16, name="wv_p", tag="wv_p")
            nc.scalar.dma_start(out=wk_p.rearrange("p (n j) -> p n j", j=128),
                                in_=wk.rearrange("(n p) j -> p n j", p=128))
            mt = wp.tile([128, 2], F32, name="mt", tag="mt")
            nc.scalar.dma_start(out=mt, in_=maskb[:, :])
            nc.scalar.dma_start(out=wq_p.rearrange("p (n j) -> p n j", j=128),
                                in_=wq.rearrange("(n p) j -> p n j", p=128))
            wk_t = [wk_p[:, d * 128:(d + 1) * 128] for d in range(ND)]
            wq_t = [wq_p[:, d * 128:(d + 1) * 128] for d in range(ND)]
            wv_t = [wv_p[:, d * 128:(d + 1) * 128] for d in range(ND)]
            # exp table preload: a 1-element exp so the ~2.7us ACT table
            # load happens during the projection lead-in, not mid-pipeline
            scr1 = wp.tile([1, 1], F32, name="scr1", tag="scr1")
            nc.scalar.activation(scr1, mt[0:1, 0:1], AF.Exp)

            # ---- persistent activations (chunked for dep granularity) ----
            def kcols(b):
                return nks[b] * 128

            def round_widths(total):
                w = []
                while total > 0:
                    w.append(min(512, total))
                    total -= w[-1]
                return w

            KTt = [[per.tile([128, w], F32R, name=f"KT{b}_{i}",
                             tag=f"KT{b}_{i}")
                    for i, w in enumerate(round_widths(kcols(b)))]
                   for b in range(2)]
            QTt = [[per.tile([128, 512], F32R, name=f"QT{b}_{r}",
                             tag=f"QT{b}_{r}") for r in range(NQS)]
                   for b in range(2)]
            Vn = [[per.tile([128, HLOC * 65], F32R, name=f"Vn{b}_{i}",
                            tag=f"Vn{b}_{i}") for i in range(nks[b])]
                  for b in range(2)]
            OcT = [[per.tile([128, 512], F32R, name=f"OcT{b}_{q}",
                             tag=f"OcT{b}_{q}") for q in range(NQS)]
                   for b in range(2)]

            def kt_slice(b, kc):
                """KT stationary slice [*, kc*128:(kc+1)*128] across tiles."""
                c0 = kc * 128
                ti, off = divmod(c0, 512)
                return KTt[b][ti][:, off:off + 128]

            def kq_round(nm, xdram, wt, OUT, b, r, dma=None):
                """One 512-wide Q projection round for batch b."""
                c0 = r * 512
                xt = xp.tile([128, ND * 512], BF16, name=f"x{nm}{b}_{r}",
                             tag="xin")
                xts = xt.rearrange("p (n s) -> p n s", n=ND)
                xdr = xdram[:, c0:c0 + 512].rearrange("(n p) s -> p n s",
                                                      p=128)
                for half in range(2):
                    dsl = slice(half * (ND // 2), (half + 1) * (ND // 2))
                    (dma or nc.sync).dma_start(out=xts[:, dsl, :],
                                               in_=xdr[:, dsl, :])
                pt = pp.tile([128, 512], F32, name=f"p{nm}{b}_{r}", tag="pp")
                for d in range(ND):
                    nc.tensor.matmul(
                        pt, wt[d], xts[:, d, :],
                        start=(d == 0), stop=(d == ND - 1))
                nc.vector.tensor_copy(OUT, pt)

            def k_rounds(b):
                """K^T projection for all nk_b k-chunks of batch b."""
                w = kcols(b)
                xkt = kvp.tile([128, ND * w], BF16, name=f"xk{b}", tag="xkv")
                xks = xkt.rearrange("p (n s) -> p n s", n=ND)
                xkd = xk[b].rearrange("(n p) s -> p n s", p=128)
                for half in range(2):
                    dsl = slice(half * (ND // 2), (half + 1) * (ND // 2))
                    nc.gpsimd.dma_start(out=xks[:, dsl, :],
                                        in_=xkd[:, dsl, :])
                for i, rw in enumerate(round_widths(w)):
                    c0 = i * 512
                    pt = pp.tile([128, rw], F32, name=f"pk{b}_{i}", tag="pp")
                    for d in range(ND):
                        nc.tensor.matmul(
                            pt, wk_t[d], xks[:, d, c0:c0 + rw],
                            start=(d == 0), stop=(d == ND - 1))
                    nc.vector.tensor_copy(KTt[b][i], pt)

            wv_loaded = [False]

            def v_round(b):
                """V natural-layout projection for all k-chunks of batch b."""
                if not wv_loaded[0]:
                    wv_loaded[0] = True
                    nc.gpsimd.dma_start(
                        out=wv_p.rearrange("p (n j) -> p n j", j=128),
                        in_=wv.rearrange("(n p) j -> p n j", p=128))
                w = kcols(b)
                xvt = kvp.tile([128, ND * w], BF16, name=f"xv{b}", tag="xkv")
                nc.gpsimd.dma_start(
                    out=xvt.rearrange("p (n s) -> p n s", n=ND),
                    in_=xv[b].rearrange("(n p) s -> p n s", p=128))
                xvs = xvt.rearrange("p (n s) -> p n s", n=ND)
                for kc in range(nks[b]):
                    pt = pp.tile([128, 128], F32, name=f"pv{b}_{kc}",
                                 tag="pp")
                    for d in range(ND):
                        nc.tensor.matmul(
                            pt, xvs[:, d, kc * 128:(kc + 1) * 128], wv_t[d],
                            start=(d == 0), stop=(d == ND - 1))
                    vt = Vn[b][kc]
                    vspl = vt.rearrange("p (h x) -> p h x", x=65)
                    nc.vector.memset(vspl[:, :, 64:65].bitcast(F32), 1.0)
                    nc.vector.tensor_copy(
                        vspl[:, :, 0:64],
                        pt.rearrange("p (h j) -> p h j", j=64))

            ncopy = [0]

            def spread_copy(dst, src, engines):
                # round-robin big PSUM->SBUF copies across the given engines
                eng = engines[ncopy[0] % len(engines)]
                ncopy[0] += 1
                if eng is nc.scalar:
                    eng.copy(dst, src)
                else:
                    eng.tensor_copy(dst, src)

            fills = []
            in_attn = [False]

            def pop_fill(n=1):
                for _ in range(n):
                    if fills:
                        fills.pop(0)()

            def flush_fills():
                while fills:
                    fills.pop(0)()

            def attention(b, h, qw, un_act=False):
                """One head, one 1024-wide q window of batch b.

                Scores run one kc ahead of AV so the in-order PE queue is
                never parked on an exp dependency; fill thunks (projection
                rounds / wo chunks) are drained between kc steps.
                """
                nk = nks[b]
                in_attn[0] = True
                hoff = h * 64
                pots = [po.tile([65, 512], F32, name=f"pot{b}_{h}_{qw}_{qh}",
                                tag="po") for qh in range(2)]
                ets = [None] * nk

                def scores_exp(kc):
                    pst = psc.tile([128, 1024], F32,
                                   name=f"pst{b}_{h}_{qw}_{kc}", tag="psc")
                    for qh in range(2):
                        nc.tensor.matmul(
                            pst[:, qh * 512:(qh + 1) * 512],
                            kt_slice(b, kc)[hoff:hoff + 64, :],
                            QTt[b][2 * qw + qh][hoff:hoff + 64, :],
                            start=True, stop=True)
                    et = ep.tile([128, 1024], F32R,
                                 name=f"et{b}_{h}_{qw}_{kc}", tag="et")
                    bias = mt[:, b:b + 1] if kc == nk - 1 else 0.0
                    nc.scalar.activation(et, pst, AF.Exp,
                                         bias=bias, scale=float(SCALE))
                    ets[kc] = et

                scores_exp(0)
                for kc in range(nk):
                    if kc + 1 < nk:
                        scores_exp(kc + 1)
                    for qh in range(2):
                        nc.tensor.matmul(
                            pots[qh],
                            Vn[b][kc][:, h * 65:h * 65 + 65],
                            ets[kc][:, qh * 512:(qh + 1) * 512],
                            start=(kc == 0), stop=(kc == nk - 1))
                    pop_fill(2 if len(fills) >= fill_slots[0] else 1)
                    fill_slots[0] = max(1, fill_slots[0] - 1)
                # 1/den can start as soon as the denominator rows are
                # staged; un copies (DVE+ACT) drain pots for Pool's muls
                # (Pool cannot read PSUM)
                # reciprocal straight off the PSUM denominator rows
                # (partition 64 is 32-aligned, so the access is legal) --
                # skips a staging copy and shortens the normalize chain
                rrow = rbp.tile([1, 1024], F32, name=f"rr{b}{h}{qw}",
                                tag="tmp1")
                for qh in range(2):
                    nc.vector.reciprocal(rrow[:, qh * 512:(qh + 1) * 512],
                                         pots[qh][64:65, :])
                rb = rbp.tile([64, 1024], F32, name=f"rb{b}{h}{qw}", tag="rb")
                nc.gpsimd.partition_broadcast(rb, rrow[0:1, :])
                uns = []
                for qh in range(2):
                    un = rbp.tile([64, 512], F32, name=f"un{b}{h}{qw}{qh}",
                                  tag="un")
                    if (qh == 1) or un_act:
                        nc.scalar.copy(un, pots[qh][0:64, :])
                    else:
                        nc.vector.tensor_copy(un, pots[qh][0:64, :])
                    uns.append(un)
                for qh in range(2):
                    nc.gpsimd.tensor_mul(
                        OcT[b][2 * qw + qh][hoff:hoff + 64, :],
                        uns[qh], rb[:, qh * 512:(qh + 1) * 512])
                in_attn[0] = False

            fill_slots = [0]

            nwo = [0]

            def wo_chunk(b, st, use_psc=False):
                """One 128-row output chunk of batch b's partial product."""
                qs, i = divmod(st, 4)
                ot = outp.tile([128, 1024], BF16, name=f"ot{b}_{st}",
                               tag="ot")
                if use_psc:
                    pt = psc.tile([128, 1024], F32, name=f"pw{b}_{st}",
                                  tag="psc")
                    for odh in range(2):
                        nc.tensor.matmul(
                            pt[:, odh * 512:(odh + 1) * 512],
                            OcT[b][qs][:, i * 128:(i + 1) * 128],
                            wo_t[:, odh * 512:(odh + 1) * 512],
                            start=True, stop=True)
                    s6 = nwo[0] % 6
                    if s6 in ((1, 4) if in_attn[0] else (1, 2, 4, 5)):
                        nc.scalar.copy(ot, pt)
                    else:
                        nc.vector.tensor_copy(ot, pt)
                    nwo[0] += 1
                    eng = nc.sync if (st % 2 == 0) else nc.gpsimd
                    eng.dma_start(
                        out=out[b][st * 128:(st + 1) * 128, :], in_=ot)
                    return
                for odh in range(2):
                    pt = pp.tile([128, 512], F32, name=f"pw{b}_{st}_{odh}",
                                 tag="pp")
                    nc.tensor.matmul(
                        pt,
                        OcT[b][qs][:, i * 128:(i + 1) * 128],
                        wo_t[:, odh * 512:(odh + 1) * 512],
                        start=True, stop=True)
                    # exp saturates ACT inside attention windows: drain
                    # on DVE there, alternate DVE/ACT outside them
                    s6 = nwo[0] % 6
                    if s6 in ((1, 4) if in_attn[0] else (1, 2, 4, 5)):
                        nc.scalar.copy(ot[:, odh * 512:(odh + 1) * 512], pt)
                    else:
                        nc.vector.tensor_copy(
                            ot[:, odh * 512:(odh + 1) * 512], pt)
                    nwo[0] += 1
                eng = nc.sync if (st % 2 == 0) else nc.gpsimd
                eng.dma_start(
                    out=out[b][st * 128:(st + 1) * 128, :], in_=ot)

            # ---- emission schedule ----
            k_rounds(0)
            kq_round("q", xq[0], wq_t, QTt[0][0], 0, 0)
            v_round(0)
            kq_round("q", xq[0], wq_t, QTt[0][1], 0, 1)

            wo_p = wp.tile([128, D], F32R, name="wo_p", tag="wo_p")
            wo_t = wo_p

            # b0 attention, PE fed by remaining projection thunks
            fills.extend([
                lambda: kq_round("q", xq[0], wq_t, QTt[0][2], 0, 2),
                lambda: kq_round("q", xq[0], wq_t, QTt[0][3], 0, 3),
                lambda: k_rounds(1),
                lambda: v_round(1),
                lambda: kq_round("q", xq[1], wq_t, QTt[1][0], 1, 0),
                lambda: kq_round("q", xq[1], wq_t, QTt[1][1], 1, 1, nc.gpsimd),
                lambda: kq_round("q", xq[1], wq_t, QTt[1][2], 1, 2),
                lambda: kq_round("q", xq[1], wq_t, QTt[1][3], 1, 3, nc.gpsimd),
            ])
            fill_slots[0] = 4 * nks[0]
            attention(0, 0, 0)
            attention(0, 1, 0)
            nc.sync.dma_start(out=wo_p, in_=wo[:, :])
            attention(0, 0, 1)
            attention(0, 1, 1)
            flush_fills()

            # b1 attention, PE fed by wo chunks; wo(1, qs) appended once its
            # OcT chunks' producers are emitted.
            fill_slots[0] = 4 * nks[1]
            for st in range(16):
                fills.append(lambda st=st: wo_chunk(0, st))
            attention(1, 0, 0)
            attention(1, 1, 0)
            for st in range(8):
                fills.append(lambda st=st: wo_chunk(1, st))
            attention(1, 0, 1, un_act=True)
            attention(1, 1, 1, un_act=True)
            flush_fills()
            for j, st in enumerate(range(8, 16)):
                wo_chunk(1, st, use_psc=(j % 2 == 0))
    nc.compile()
    return nc


def _get_nc(nk0, nk1):
    key = (nk0, nk1)
    if key not in _cached:
        _cached[key] = _build(nk0, nk1)
    return _cached[key]


def kernel(queries, keys, values, valid_lens, Wq, Wk, Wv, Wo, **kwargs):
    queries = np.asarray(queries, dtype=np.float32)
    keys = np.asarray(keys, dtype=np.float32)
    values = np.asarray(values, dtype=np.float32)
    Wq = np.asarray(Wq, dtype=np.float32)
    Wk = np.asarray(Wk, dtype=np.float32)
    Wv = np.asarray(Wv, dtype=np.float32)
    Wo = np.asarray(Wo, dtype=np.float32)
    vls = np.asarray(valid_lens).astype(np.int64)
    B = queries.shape[0]
    assert B == 2 and queries.shape[1:] == (S, D), \
        f"kernel compiled for (2, {S}, {D}), got {queries.shape}"

    nks = [int(min(16, max(1, -(-int(vls[b]) // 128)))) for b in range(B)]
    nc = _get_nc(nks[0], nks[1])

    bf16 = ml_dtypes.bfloat16
    xqs, xks, xvs, mks = [], [], [], []
    for b in range(B):
        vl = int(vls[b])
        nk = nks[b]
        qb = queries[b]
        if vl <= 0:
            # reference: fully-masked row -> softmax of constant -> uniform.
            # (cannot happen with this reference's randint(1, S+1) bounds)
            qb = np.zeros_like(qb)
            mk = np.zeros(128, np.float32)
        else:
            pos = (nk - 1) * 128 + np.arange(128)
            mk = np.where(pos < vl, 0.0, MASK_VALUE).astype(np.float32)
        mks.append(mk)
        xqs.append(np.ascontiguousarray(qb.T).astype(bf16))
        xks.append(np.ascontiguousarray(keys[b].T[:, :nk * 128]).astype(bf16))
        xvs.append(np.ascontiguousarray(values[b].T[:, :nk * 128]).astype(bf16))
    mkt = np.ascontiguousarray(np.stack(mks, axis=1))  # [128, 2]

    in_maps = []
    for c in range(8):
        sl = slice(c * 128, (c + 1) * 128)
        in_maps.append({
            "xq0": xqs[0], "xq1": xqs[1],
            "xk0": xks[0], "xk1": xks[1],
            "xv0": xvs[0], "xv1": xvs[1],
            "wq": np.ascontiguousarray(Wq[:, sl]).astype(bf16),
            "wk": np.ascontiguousarray(Wk[:, sl]).astype(bf16),
            "wv": np.ascontiguousarray(Wv[:, sl]).astype(bf16),
            "wo": np.ascontiguousarray(Wo[sl, :]),
            "maskb": mkt,
        })

    res = run_bass_kernel_spmd(nc, in_maps, core_ids=list(range(8)), **kwargs)
    global LAST_RESULTS
    LAST_RESULTS = res

    outp = np.zeros((B, S, D), np.float32)
    for b in range(B):
        acc = res.results[0][f"out{b}"].astype(np.float32)
        for c in range(1, 8):
            acc = acc + res.results[c][f"out{b}"].astype(np.float32)
        outp[b] = acc
    return outp


# revision 8
# speedup vs baseline: 2.3993x; 1.0122x over previous
"""Multi-head attention (16 heads, D=1024, B=2, S=2048) on 8 Trainium2 cores.

Sharding v2: head-wise tensor parallel — each core owns 2 heads (128 of the
1024 projection dims) and processes BOTH batches.  Per-core partial outputs
(full [2, 2048, 1024] shape through its 128 rows of Wo) are summed on host.

Key optimization vs v1: `valid_lens` is known at kernel-build time and masks
all scores at k >= vl to exp(-1e6) == 0 exactly, so k-chunks beyond
ceil(vl/128) contribute nothing to numerator or denominator.  The kernel is
compiled per (nk0, nk1) = ceil(vl/128) and never computes the masked
K/V projections, scores, exps, or AV products.  With vl=[288, 576] that cuts
attention work 4x and K/V projection work 3.2x, and head-wise sharding keeps
all 8 cores perfectly balanced (each sees both batches).

Device layout (per core, per batch b):
  X^T (feature-major, bf16) --(Wq/Wk stationary)--> QT/KT [j=128 dims, s]
  KT.T @ QT = scores^T [k, q] --exp(scale*x + mask)--> E [k, q]  (f32r)
  Vn natural [k, 2*65] (64 dims + ones col per head):
    Vn_h.T @ E accumulates attn-weighted V AND the softmax denominator
    (row 64) in one PSUM accumulation group.
  normalize: OcT[h*64:, q] = pots[0:64] * broadcast(1/pots[64])
  wo: OcT chunk [128, 128] stationary x Wo rows [128, 1024] -> out partial.

All attention matmuls run f32r at 1 cycle/row (free dim >= 256; the V/K-proj
tails with free 128 are bf16-input).  Outputs are written bf16 to halve the
output DMA; host accumulates the 8 partials in f32.
"""
import ml_dtypes
import numpy as np

import concourse.bacc as bacc
import concourse.mybir as mybir
import concourse.tile as tile
from concourse.bass_utils import run_bass_kernel_spmd

F32 = mybir.dt.float32
F32R = mybir.dt.float32r
BF16 = mybir.dt.bfloat16
AF = mybir.ActivationFunctionType

S = 2048          # sequence length
D = 1024          # model dim
HLOC = 2          # heads per core
HD = 64           # head dim
SCALE = 1.0 / np.sqrt(32.0)   # reference bug: d_k = B*H = 32
MASK_VALUE = -1.0e6

ND = 8            # d chunks of 128 (contraction for projections)
NQS = 4           # q chunks of 512 per batch (OcT/QT chunk granularity)

_cached = {}
LAST_RESULTS = None


def _build(nk0, nk1):
    nks = [nk0, nk1]
    nc = bacc.Bacc("TRN2", target_bir_lowering=False, debug=False,
                   num_swdge_queues=4)

    xq = [nc.dram_tensor(f"xq{b}", [D, S], BF16, kind="ExternalInput")
          for b in range(2)]
    xk = [nc.dram_tensor(f"xk{b}", [D, nks[b] * 128], BF16,
                         kind="ExternalInput") for b in range(2)]
    xv = [nc.dram_tensor(f"xv{b}", [D, nks[b] * 128], BF16,
                         kind="ExternalInput") for b in range(2)]
    wq = nc.dram_tensor("wq", [D, 128], BF16, kind="ExternalInput")
    wk = nc.dram_tensor("wk", [D, 128], BF16, kind="ExternalInput")
    wv = nc.dram_tensor("wv", [D, 128], BF16, kind="ExternalInput")
    wo = nc.dram_tensor("wo", [128, D], F32R, kind="ExternalInput")
    maskb = nc.dram_tensor("maskb", [128, 2], F32, kind="ExternalInput")
    out = [nc.dram_tensor(f"out{b}", [S, D], BF16, kind="ExternalOutput")
           for b in range(2)]

    with tile.TileContext(nc) as tc:
        with tc.tile_pool(name="wp", bufs=1) as wp, \
             tc.tile_pool(name="per", bufs=1) as per, \
             tc.tile_pool(name="xp", bufs=3) as xp, \
             tc.tile_pool(name="kvp", bufs=2) as kvp, \
             tc.tile_pool(name="ep", bufs=5) as ep, \
             tc.tile_pool(name="rbp", bufs=7) as rbp, \
             tc.tile_pool(name="outp", bufs=8) as outp, \
             tc.tile_pool(name="pp", bufs=2, space="PSUM") as pp, \
             tc.tile_pool(name="po", bufs=2, space="PSUM") as po, \
             tc.tile_pool(name="psc", bufs=2, space="PSUM") as psc:

            # ---- mask + packed projection weights ----
            # (wk first on SP and xk0 first on Pool: the K projection is the
            # head of the whole pipeline)
            wk_p = wp.tile([128, ND * 128], BF16, name="wk_p", tag="wk_p")
            wq_p = wp.tile([128, ND * 128], BF16, name="wq_p", tag="wq_p")
            wv_p = wp.tile([128, ND * 128], BF16, name="wv_p", tag="wv_p")
            nc.scalar.dma_start(out=wk_p.rearrange("p (n j) -> p n j", j=128),
                                in_=wk.rearrange("(n p) j -> p n j", p=128))
            mt = wp.tile([128, 2], F32, name="mt", tag="mt")
            nc.scalar.dma_start(out=mt, in_=maskb[:, :])
            nc.scalar.dma_start(out=wq_p.rearrange("p (n j) -> p n j", j=128),
                                in_=wq.rearrange("(n p) j -> p n j", p=128))
            wk_t = [wk_p[:, d * 128:(d + 1) * 128] for d in range(ND)]
            wq_t = [wq_p[:, d * 128:(d + 1) * 128] for d in range(ND)]
            wv_t = [wv_p[:, d * 128:(d + 1) * 128] for d in range(ND)]
            # exp table preload: a 1-element exp so the ~2.7us ACT table
            # load happens during the projection lead-in, not mid-pipeline
            scr1 = wp.tile([1, 1], F32, name="scr1", tag="scr1")
            nc.scalar.activation(scr1, mt[0:1, 0:1], AF.Exp)

            # ---- persistent activations (chunked for dep granularity) ----
            def kcols(b):
                return nks[b] * 128

            def round_widths(total):
                w = []
                while total > 0:
                    w.append(min(512, total))
                    total -= w[-1]
                return w

            KTt = [[per.tile([128, w], F32R, name=f"KT{b}_{i}",
                             tag=f"KT{b}_{i}")
                    for i, w in enumerate(round_widths(kcols(b)))]
                   for b in range(2)]
            QTt = [[per.tile([128, 512], F32R, name=f"QT{b}_{r}",
                             tag=f"QT{b}_{r}") for r in range(NQS)]
                   for b in range(2)]
            Vn = [[per.tile([128, HLOC * 65], F32R, name=f"Vn{b}_{i}",
                            tag=f"Vn{b}_{i}") for i in range(nks[b])]
                  for b in range(2)]
            OcT = [[per.tile([128, 512], F32R, name=f"OcT{b}_{q}",
                             tag=f"OcT{b}_{q}") for q in range(NQS)]
                   for b in range(2)]

            def kt_slice(b, kc):
                """KT stationary slice [*, kc*128:(kc+1)*128] across tiles."""
                c0 = kc * 128
                ti, off = divmod(c0, 512)
                return KTt[b][ti][:, off:off + 128]

            def kq_round(nm, xdram, wt, OUT, b, r, dma=None):
                """One 512-wide Q projection round for batch b."""
                c0 = r * 512
                xt = xp.tile([128, ND * 512], BF16, name=f"x{nm}{b}_{r}",
                             tag="xin")
                xts = xt.rearrange("p (n s) -> p n s", n=ND)
                xdr = xdram[:, c0:c0 + 512].rearrange("(n p) s -> p n s",
                                                      p=128)
                for half in range(2):
                    dsl = slice(half * (ND // 2), (half + 1) * (ND // 2))
                    (dma or nc.sync).dma_start(out=xts[:, dsl, :],
                                               in_=xdr[:, dsl, :])
                pt = pp.tile([128, 512], F32, name=f"p{nm}{b}_{r}", tag="pp")
                for d in range(ND):
                    nc.tensor.matmul(
                        pt, wt[d], xts[:, d, :],
                        start=(d == 0), stop=(d == ND - 1))
                nc.vector.tensor_copy(OUT, pt)

            def k_rounds(b):
                """K^T projection for all nk_b k-chunks of batch b."""
                w = kcols(b)
                xkt = kvp.tile([128, ND * w], BF16, name=f"xk{b}", tag="xkv")
                xks = xkt.rearrange("p (n s) -> p n s", n=ND)
                xkd = xk[b].rearrange("(n p) s -> p n s", p=128)
                for half in range(2):
                    dsl = slice(half * (ND // 2), (half + 1) * (ND // 2))
                    nc.gpsimd.dma_start(out=xks[:, dsl, :],
                                        in_=xkd[:, dsl, :])
                for i, rw in enumerate(round_widths(w)):
                    c0 = i * 512
                    pt = pp.tile([128, rw], F32, name=f"pk{b}_{i}", tag="pp")
                    for d in range(ND):
                        nc.tensor.matmul(
                            pt, wk_t[d], xks[:, d, c0:c0 + rw],
                            start=(d == 0), stop=(d == ND - 1))
                    nc.vector.tensor_copy(KTt[b][i], pt)

            wv_loaded = [False]

            def v_round(b):
                """V natural-layout projection for all k-chunks of batch b."""
                if not wv_loaded[0]:
                    wv_loaded[0] = True
                    nc.gpsimd.dma_start(
                        out=wv_p.rearrange("p (n j) -> p n j", j=128),
                        in_=wv.rearrange("(n p) j -> p n j", p=128))
                w = kcols(b)
                xvt = kvp.tile([128, ND * w], BF16, name=f"xv{b}", tag="xkv")
                nc.gpsimd.dma_start(
                    out=xvt.rearrange("p (n s) -> p n s", n=ND),
                    in_=xv[b].rearrange("(n p) s -> p n s", p=128))
                xvs = xvt.rearrange("p (n s) -> p n s", n=ND)
                for kc in range(nks[b]):
                    pt = pp.tile([128, 128], F32, name=f"pv{b}_{kc}",
                                 tag="pp")
                    for d in range(ND):
                        nc.tensor.matmul(
                            pt, xvs[:, d, kc * 128:(kc + 1) * 128], wv_t[d],
                            start=(d == 0), stop=(d == ND - 1))
                    vt = Vn[b][kc]
                    vspl = vt.rearrange("p (h x) -> p h x", x=65)
                    nc.vector.memset(vspl[:, :, 64:65].bitcast(F32), 1.0)
                    nc.vector.tensor_copy(
                        vspl[:, :, 0:64],
                        pt.rearrange("p (h j) -> p h j", j=64))

            ncopy = [0]

            def spread_copy(dst, src, engines):
                # round-robin big PSUM->SBUF copies across the given engines
                eng = engines[ncopy[0] % len(engines)]
                ncopy[0] += 1
                if eng is nc.scalar:
                    eng.copy(dst, src)
                else:
                    eng.tensor_copy(dst, src)

            fills = []
            in_attn = [False]

            def pop_fill(n=1):
                for _ in range(n):
                    if fills:
                        fills.pop(0)()

            def flush_fills():
                while fills:
                    fills.pop(0)()

            def attention(b, h, qw, un_act=False):
                """One head, one 1024-wide q window of batch b.

                Scores run one kc ahead of AV so the in-order PE queue is
                never parked on an exp dependency; fill thunks (projection
                rounds / wo chunks) are drained between kc steps.
                """
                nk = nks[b]
                in_attn[0] = True
                hoff = h * 64
                pots = [po.tile([65, 512], F32, name=f"pot{b}_{h}_{qw}_{qh}",
                                tag="po") for qh in range(2)]
                ets = [None] * nk

                def scores_exp(kc):
                    pst = psc.tile([128, 1024], F32,
                                   name=f"pst{b}_{h}_{qw}_{kc}", tag="psc")
                    for qh in range(2):
                        nc.tensor.matmul(
                            pst[:, qh * 512:(qh + 1) * 512],
                            kt_slice(b, kc)[hoff:hoff + 64, :],
                            QTt[b][2 * qw + qh][hoff:hoff + 64, :],
                            start=True, stop=True)
                    et = ep.tile([128, 1024], F32R,
                                 name=f"et{b}_{h}_{qw}_{kc}", tag="et")
                    bias = mt[:, b:b + 1] if kc == nk - 1 else 0.0
                    nc.scalar.activation(et, pst, AF.Exp,
                                         bias=bias, scale=float(SCALE))
                    ets[kc] = et

                scores_exp(0)
                for kc in range(nk):
                    if kc + 1 < nk:
                        scores_exp(kc + 1)
                    for qh in range(2):
                        nc.tensor.matmul(
                            pots[qh],
                            Vn[b][kc][:, h * 65:h * 65 + 65],
                            ets[kc][:, qh * 512:(qh + 1) * 512],
                            start=(kc == 0), stop=(kc == nk - 1))
                    pop_fill(2 if len(fills) >= fill_slots[0] else 1)
                    fill_slots[0] = max(1, fill_slots[0] - 1)
                # 1/den can start as soon as the denominator rows are
                # staged; un copies (DVE+ACT) drain pots for Pool's muls
                # (Pool cannot read PSUM)
                # reciprocal straight off the PSUM denominator rows
                # (partition 64 is 32-aligned, so the access is legal) --
                # skips a staging copy and shortens the normalize chain
                rrow = rbp.tile([1, 1024], F32, name=f"rr{b}{h}{qw}",
                                tag="tmp1")
                for qh in range(2):
                    nc.vector.reciprocal(rrow[:, qh * 512:(qh + 1) * 512],
                                         pots[qh][64:65, :])
                rb = rbp.tile([64, 1024], F32, name=f"rb{b}{h}{qw}", tag="rb")
                nc.gpsimd.partition_broadcast(rb, rrow[0:1, :])
                uns = []
                for qh in range(2):
                    un = rbp.tile([64, 512], F32, name=f"un{b}{h}{qw}{qh}",
                                  tag="un")
                    if (qh == 1) or un_act:
                        nc.scalar.copy(un, pots[qh][0:64, :])
                    else:
                        nc.vector.tensor_copy(un, pots[qh][0:64, :])
                    uns.append(un)
                for qh in range(2):
                    nc.gpsimd.tensor_mul(
                        OcT[b][2 * qw + qh][hoff:hoff + 64, :],
                        uns[qh], rb[:, qh * 512:(qh + 1) * 512])
                in_attn[0] = False

            fill_slots = [0]

            nwo = [0]

            def wo_chunk(b, st, use_psc=False):
                """One 128-row output chunk of batch b's partial product."""
                qs, i = divmod(st, 4)
                ot = outp.tile([128, 1024], BF16, name=f"ot{b}_{st}",
                               tag="ot")
                if use_psc:
                    pt = psc.tile([128, 1024], F32, name=f"pw{b}_{st}",
                                  tag="psc")
                    for odh in range(2):
                        nc.tensor.matmul(
                            pt[:, odh * 512:(odh + 1) * 512],
                            OcT[b][qs][:, i * 128:(i + 1) * 128],
                            wo_t[:, odh * 512:(odh + 1) * 512],
                            start=True, stop=True)
                    s6 = nwo[0] % 6
                    if s6 in ((1,) if in_attn[0] else (1, 2, 4, 5)):
                        nc.scalar.copy(ot, pt)
                    else:
                        nc.vector.tensor_copy(ot, pt)
                    nwo[0] += 1
                    eng = nc.sync if (st % 2 == 0) else nc.gpsimd
                    eng.dma_start(
                        out=out[b][st * 128:(st + 1) * 128, :], in_=ot)
                    return
                for odh in range(2):
                    pt = pp.tile([128, 512], F32, name=f"pw{b}_{st}_{odh}",
                                 tag="pp")
                    nc.tensor.matmul(
                        pt,
                        OcT[b][qs][:, i * 128:(i + 1) * 128],
                        wo_t[:, odh * 512:(odh + 1) * 512],
                        start=True, stop=True)
                    # exp saturates ACT inside attention windows: drain
                    # on DVE there, alternate DVE/ACT outside them
                    s6 = nwo[0] % 6
                    if s6 in ((1,) if in_attn[0] else (1, 2, 4, 5)):
                        nc.scalar.copy(ot[:, odh * 512:(odh + 1) * 512], pt)
                    else:
                        nc.vector.tensor_copy(
                            ot[:, odh * 512:(odh + 1) * 512], pt)
                    nwo[0] += 1
                eng = nc.sync if (st % 2 == 0) else nc.gpsimd
                eng.dma_start(
                    out=out[b][st * 128:(st + 1) * 128, :], in_=ot)

            # ---- emission schedule ----
            k_rounds(0)
            kq_round("q", xq[0], wq_t, QTt[0][0], 0, 0)
            v_round(0)
            kq_round("q", xq[0], wq_t, QTt[0][1], 0, 1)

            wo_p = wp.tile([128, D], F32R, name="wo_p", tag="wo_p")
            wo_t = wo_p

            # b0 attention, PE fed by remaining projection thunks
            fills.extend([
                lambda: kq_round("q", xq[0], wq_t, QTt[0][2], 0, 2),
                lambda: kq_round("q", xq[0], wq_t, QTt[0][3], 0, 3),
                lambda: k_rounds(1),
                lambda: v_round(1),
                lambda: kq_round("q", xq[1], wq_t, QTt[1][0], 1, 0),
                lambda: kq_round("q", xq[1], wq_t, QTt[1][1], 1, 1, nc.gpsimd),
                lambda: kq_round("q", xq[1], wq_t, QTt[1][2], 1, 2),
                lambda: kq_round("q", xq[1], wq_t, QTt[1][3], 1, 3, nc.gpsimd),
            ])
            fill_slots[0] = 4 * nks[0]
            attention(0, 0, 0)
            attention(0, 1, 0)
            nc.sync.dma_start(out=wo_p, in_=wo[:, :])
            attention(0, 0, 1)
            attention(0, 1, 1)
            flush_fills()

            # b1 attention, PE fed by wo chunks; wo(1, qs) appended once its
            # OcT chunks' producers are emitted.
            fill_slots[0] = 4 * nks[1]
            for st in range(16):
                fills.append(lambda st=st: wo_chunk(0, st))
            attention(1, 0, 0)
            attention(1, 1, 0)
            for st in range(8):
                fills.append(lambda st=st: wo_chunk(1, st))
            attention(1, 0, 1, un_act=True)
            attention(1, 1, 1, un_act=True)
            flush_fills()
            for j, st in enumerate(range(8, 16)):
                wo_chunk(1, st, use_psc=(j % 2 == 0))
    nc.compile()
    return nc


def _get_nc(nk0, nk1):
    key = (nk0, nk1)
    if key not in _cached:
        _cached[key] = _build(nk0, nk1)
    return _cached[key]


def kernel(queries, keys, values, valid_lens, Wq, Wk, Wv, Wo, **kwargs):
    queries = np.asarray(queries, dtype=np.float32)
    keys = np.asarray(keys, dtype=np.float32)
    values = np.asarray(values, dtype=np.float32)
    Wq = np.asarray(Wq, dtype=np.float32)
    Wk = np.asarray(Wk, dtype=np.float32)
    Wv = np.asarray(Wv, dtype=np.float32)
    Wo = np.asarray(Wo, dtype=np.float32)
    vls = np.asarray(valid_lens).astype(np.int64)
    B = queries.shape[0]
    assert B == 2 and queries.shape[1:] == (S, D), \
        f"kernel compiled for (2, {S}, {D}), got {queries.shape}"

    nks = [int(min(16, max(1, -(-int(vls[b]) // 128)))) for b in range(B)]
    nc = _get_nc(nks[0], nks[1])

    bf16 = ml_dtypes.bfloat16
    xqs, xks, xvs, mks = [], [], [], []
    for b in range(B):
        vl = int(vls[b])
        nk = nks[b]
        qb = queries[b]
        if vl <= 0:
            # reference: fully-masked row -> softmax of constant -> uniform.
            # (cannot happen with this reference's randint(1, S+1) bounds)
            qb = np.zeros_like(qb)
            mk = np.zeros(128, np.float32)
        else:
            pos = (nk - 1) * 128 + np.arange(128)
            mk = np.where(pos < vl, 0.0, MASK_VALUE).astype(np.float32)
        mks.append(mk)
        xqs.append(np.ascontiguousarray(qb.T).astype(bf16))
        xks.append(np.ascontiguousarray(keys[b].T[:, :nk * 128]).astype(bf16))
        xvs.append(np.ascontiguousarray(values[b].T[:, :nk * 128]).astype(bf16))
    mkt = np.ascontiguousarray(np.stack(mks, axis=1))  # [128, 2]

    in_maps = []
    for c in range(8):
        sl = slice(c * 128, (c + 1) * 128)
        in_maps.append({
            "xq0": xqs[0], "xq1": xqs[1],
            "xk0": xks[0], "xk1": xks[1],
            "xv0": xvs[0], "xv1": xvs[1],
            "wq": np.ascontiguousarray(Wq[:, sl]).astype(bf16),
            "wk": np.ascontiguousarray(Wk[:, sl]).astype(bf16),
            "wv": np.ascontiguousarray(Wv[:, sl]).astype(bf16),
            "wo": np.ascontiguousarray(Wo[sl, :]),
            "maskb": mkt,
        })

    res = run_bass_kernel_spmd(nc, in_maps, core_ids=list(range(8)), **kwargs)
    global LAST_RESULTS
    LAST_RESULTS = res

    outp = np.zeros((B, S, D), np.float32)
    for b in range(B):
        acc = res.results[0][f"out{b}"].astype(np.float32)
        for c in range(1, 8):
            acc = acc + res.results[c][f"out{b}"].astype(np.float32)
        outp[b] = acc
    return outp
